# revision 1
# baseline (speedup 1.0000x reference)
"""Distributed Trainium2 kernel for a GATv2 layer + BN + global-mean-pool + classifier.

Math (reference, heads=1):
    xl = x@Wl + bl ; xr = x@Wr + br
    logit_e = att . leaky_relu(xl[src_e] + xr[dst_e], 0.2)
    a_e     = segment_softmax(logit_e over dst)
    out_i   = sum_{e: dst=i} a_e * xl[src_e] ; out = out + bias1
    h       = BN(out) ; g = mean_i h ; y = softmax(g@Wc + bc)

The output is a global mean over nodes and BN is affine per feature, so
per-node outputs never materialize:
    y = softmax( ((S/N)*A + B) @ Wc + bc ),  S = sum_e a_e * xl[src_e],
    A = gamma/sqrt(var+eps), B = (bias1 - mu)*A + beta.

Attention weights v = att are folded into the gather tables host-side:
    v_f * lrelu(z_f) = sign_f * lrelu(|v_f| z_f)
with features permuted so positive-sign features occupy columns [0,PP).
The gathered table is xg = perm(|v| (.) xl); since |v|>0 this is inverted
through the head constants (A' = A[perm]/(N |v|), Wc' = Wc[perm]), so the
same gathered rows serve both the logits and the weighted sum.

Distribution over 8 cores: nodes sharded contiguously; edges sharded by dst
and packed (whole per-dst segments, one partition each) into a
[128 partitions x L slots] grid. Per-edge logits come from bulk dma_gather
(int16 indices -> two half-tables, invalid side pointing at a -1e30 dummy
row, merged with one elementwise max). Segment softmax = forward masked
scan + reversed masked max-scan. A second gather pass computes
partial = sum_slots w * xg[src] in PSUM; AllReduce + a tiny head finish.
"""

import math
import os

import ml_dtypes
import numpy as np

import concourse.bass as bass
import concourse.bacc as bacc
import concourse.mybir as mybir
import concourse.tile as tile

M = 8  # cores
F = 128
NCLS = 5
BN_EPS = 1e-5

BF16 = ml_dtypes.bfloat16


def _wrap_idx(seq):
    """[N] int array -> [128, N//16] int16 wrap layout (16-partition groups,
    replicated across the 8 gpsimd cores)."""
    n = seq.shape[0]
    assert n % 16 == 0
    w = seq.reshape(n // 16, 16).T.astype(np.int16)
    return np.tile(w, (8, 1))


def _segment_fields(sorted_key):
    n = sorted_key.shape[0]
    start = np.ones(n, bool)
    start[1:] = sorted_key[1:] != sorted_key[:-1]
    end = np.ones(n, bool)
    end[:-1] = sorted_key[1:] != sorted_key[:-1]
    idx = np.arange(n, dtype=np.int64)
    first = np.where(start, idx, 0)
    first = np.maximum.accumulate(first)
    return start, end, idx - first


def prep_host(x, edge_index, Wl, bl, Wr, br, att, bias1,
              bn_gamma, bn_beta, bn_mean, bn_var, Wc, bc):
    N = x.shape[0]
    npc = N // M
    assert npc * M == N
    NPC = ((npc + 1 + 127) // 128) * 128  # always >= 1 pad row (dummy)
    CH = NPC // 128
    NG = M * NPC
    HALF = NG // 2
    DUM = npc  # first pad row of each core's shard holds -1e30

    src = np.concatenate([edge_index[0], np.arange(N, dtype=np.int64)])
    dst = np.concatenate([edge_index[1], np.arange(N, dtype=np.int64)])

    # ---- attention folding ----
    v = np.asarray(att[0], np.float64)
    posm = v >= 0
    perm = np.argsort(~posm, kind="stable")
    PP = int(posm.sum())
    assert 0 < PP < F, f"degenerate attention sign split PP={PP}"
    absv = np.abs(v[perm])
    Wg_l = (Wl[:, perm] * absv[None, :]).astype(np.float32)
    bg_l = (bl[perm] * absv).astype(np.float32)
    Wg_r = (Wr[:, perm] * absv[None, :]).astype(np.float32)
    bg_r = (br[perm] * absv).astype(np.float32)

    nodes = np.arange(N, dtype=np.int64)
    cN = nodes // npc
    lN = nodes % npc

    # ---- dst-grid: greedy LPT node->partition packing ----
    deg = np.bincount(dst, minlength=N)
    pnode = np.zeros(N, np.int64)    # partition of each node
    soff = np.zeros(N, np.int64)     # slot offset of node's segment
    Lmax = 0
    for k in range(M):
        dk = deg[k * npc:(k + 1) * npc]
        order_n = np.argsort(-dk, kind="stable")
        loads = np.zeros(128, np.int64)
        for g in order_n:
            p = int(np.argmin(loads))
            pnode[k * npc + g] = p
            soff[k * npc + g] = loads[p]
            loads[p] += dk[g]
        Lmax = max(Lmax, int(loads.max()))
    # L must be a multiple of 128 so chunk idx sequences tile evenly
    L = ((max(Lmax, 128) + 127) // 128) * 128

    order = np.argsort(dst, kind="stable")
    ds = dst[order]
    ss = src[order]
    d_start, d_end, q = _segment_fields(ds)
    cd = ds // npc
    dls = ds - cd * npc
    pd = pnode[ds]
    t = soff[ds] + q
    assert t.max() < L

    srcrow = (ss // npc) * NPC + (ss % npc)  # global padded row
    iP = np.zeros((M, 128, L), np.int64)              # pair row (2 nodes/row)
    par = np.zeros((M, 128, L), np.float32)           # which half of the pair
    iP[cd, pd, t] = srcrow >> 1
    par[cd, pd, t] = (srcrow & 1).astype(np.float32)

    iX = np.zeros((M, 128, L), np.int64)              # xr local row
    iX[cd, pd, t] = dls
    mask_f = np.zeros((M, 128, L), np.float32)
    mask_r = np.zeros((M, 128, L), np.float32)
    mask_v = np.zeros((M, 128, L), np.float32)
    mask_f[cd, pd, t] = (~d_start).astype(np.float32)
    mask_r[cd, pd, t] = (~d_end).astype(np.float32)
    mask_v[cd, pd, t] = 1.0

    # gather index order: position i -> slot (p = i%128, t = i//128)
    def to_wrap(a):  # [128, L] -> wrap over i-sequence
        seq = a.T.reshape(-1)  # i = t*128 + p
        return _wrap_idx(seq)

    iP_w = np.stack([to_wrap(iP[k]) for k in range(M)])
    iX_w = np.stack([to_wrap(iX[k]) for k in range(M)])

    # ---- head constants (de-permuted / de-scaled) ----
    A = bn_gamma.astype(np.float64) / np.sqrt(bn_var.astype(np.float64) + BN_EPS)
    Ap = (A[perm] / (N * absv)).astype(np.float32).reshape(F, 1)
    Bp = ((bias1 - bn_mean).astype(np.float64) * A + bn_beta)[perm] \
        .astype(np.float32).reshape(F, 1)
    Wcp = Wc[perm, :].astype(np.float32)

    # ---- per-core x^T (padded, bf16) ----
    xT = np.zeros((M, 128, NPC), BF16)
    for k in range(M):
        xT[k, :, :npc] = x[k * npc:(k + 1) * npc].T.astype(BF16)

    meta = dict(NPC=NPC, CH=CH, NG=NG, L=L, PP=PP, N=N, DUM=DUM)

    in_maps = []
    for k in range(M):
        in_maps.append({
            "xT": np.ascontiguousarray(xT[k]),
            "Wgl": Wg_l.astype(BF16),
            "bgl": bg_l.reshape(1, F).astype(BF16),
            "Wgr": Wg_r.astype(BF16),
            "bgr": bg_r.reshape(1, F).astype(BF16),
            "iP": np.ascontiguousarray(iP_w[k]),
            "iX": np.ascontiguousarray(iX_w[k]),
            "par": np.ascontiguousarray(par[k]),
            "mask_f": np.ascontiguousarray(mask_f[k]),
            "mask_r": np.ascontiguousarray(mask_r[k]),
            "mask_v": np.ascontiguousarray(mask_v[k]),
            "Ap": Ap,
            "Bp": Bp,
            "Wcp": Wcp,
            "bc": bc.reshape(1, NCLS).astype(np.float32),
        })
    return in_maps, meta


def build(meta, nchunks=24):
    stage = int(os.environ.get("KERNEL_STAGE", "3"))
    NPC, CH, NG, L, PP, DUM = (meta[k] for k in ("NPC", "CH", "NG", "L", "PP", "DUM"))
    HALF = NG // 2
    while nchunks > 1 and L % (nchunks * 8):
        nchunks -= 1
    KC = L // nchunks      # slot-columns per chunk
    NI = KC * 128          # gather indices per call
    LW = (L * 128) // 16   # wrap-index array width

    dt = mybir.dt
    alu = mybir.AluOpType
    act = mybir.ActivationFunctionType
    rg = [list(range(M))]

    nc = bacc.Bacc("TRN2", target_bir_lowering=False, debug=False, num_devices=M)

    def p_in(name, shape, d):
        return nc.dram_tensor(name, shape, d, kind="ExternalInput").ap()

    xT = p_in("xT", [128, NPC], dt.bfloat16)
    Wgl = p_in("Wgl", [F, F], dt.bfloat16)
    bgl = p_in("bgl", [1, F], dt.bfloat16)
    Wgr = p_in("Wgr", [F, F], dt.bfloat16)
    bgr = p_in("bgr", [1, F], dt.bfloat16)
    iP = p_in("iP", [128, LW], dt.int16)
    iX = p_in("iX", [128, LW], dt.int16)
    par = p_in("par", [128, L], dt.float32)
    mask_f = p_in("mask_f", [128, L], dt.float32)
    mask_r = p_in("mask_r", [128, L], dt.float32)
    mask_v = p_in("mask_v", [128, L], dt.float32)
    Ap = p_in("Ap", [F, 1], dt.float32)
    Bp = p_in("Bp", [F, 1], dt.float32)
    Wcp = p_in("Wcp", [F, NCLS], dt.float32)
    bc = p_in("bc", [1, NCLS], dt.float32)
    out = nc.dram_tensor("out", [1, NCLS], dt.float32, kind="ExternalOutput").ap()

    with tile.TileContext(nc) as tc:
        with (
            tc.tile_pool(name="dram", bufs=1, space="DRAM") as dpool,
            tc.tile_pool(name="sbp", bufs=1) as sbp,
            tc.tile_pool(name="sbw", bufs=2) as sbw,
            tc.tile_pool(name="ps2", bufs=2, space="PSUM") as pp,
            tc.tile_pool(name="ps1", bufs=1, space="PSUM") as pp1,
        ):
            xg_loc = dpool.tile([NPC, F], dt.bfloat16)
            xr_loc = dpool.tile([NPC, 2 * F], dt.bfloat16)
            xg_full = dpool.tile([NG, F], dt.bfloat16, addr_space="Shared")
            xls_scr = dpool.tile([128, L * 2 * F], dt.bfloat16)
            part_loc = dpool.tile([1, F], dt.float32)
            pooled = dpool.tile([1, F], dt.float32, addr_space="Shared")

            # ---- persistent SBUF ----
            xT_sb = sbp.tile([128, NPC], dt.bfloat16)
            nc.sync.dma_start(xT_sb[:], xT)
            wt = {}
            for nm, apin, sh in (("Wgl", Wgl, [F, F]), ("bgl", bgl, [1, F]),
                                 ("Wgr", Wgr, [F, F]), ("bgr", bgr, [1, F])):
                tl = sbp.tile(sh, dt.bfloat16, tag=nm)
                nc.sync.dma_start(tl[:], apin)
                wt[nm] = tl
            ones_sb = sbp.tile([1, F], dt.bfloat16)
            nc.vector.memset(ones_sb[:], 1.0)

            iP_sb = sbp.tile([128, LW], dt.int16)
            nc.sync.dma_start(iP_sb[:], iP)
            iX_sb = sbp.tile([128, LW], dt.int16)
            nc.sync.dma_start(iX_sb[:], iX)
            par_sb = sbp.tile([128, L], dt.float32)
            nc.sync.dma_start(par_sb[:], par)
            mf_sb = sbp.tile([128, L], dt.float32)
            nc.sync.dma_start(mf_sb[:], mask_f)
            mr_sb = sbp.tile([128, L], dt.float32)
            nc.sync.dma_start(mr_sb[:], mask_r)
            mv_sb = sbp.tile([128, L], dt.float32)
            nc.sync.dma_start(mv_sb[:], mask_v)

            logits_sb = sbp.tile([128, L], dt.float32)
            if stage >= 2:
                E_sb = sbp.tile([128, L], dt.float32)
                S_sb = sbp.tile([128, L], dt.float32)
                D_sb = sbp.tile([128, L], dt.float32)
                w_sb = sbp.tile([128, L], dt.float32)
                wb_sb = sbp.tile([128, L], dt.bfloat16)
                wb1_sb = sbp.tile([128, L], dt.bfloat16)

            # ---- stage A: node tables ----
            for ci in range(CH):
                lhs = xT_sb[:, 128 * ci:128 * (ci + 1)]
                for wn, bn_ in (("Wgl", "bgl"), ("Wgr", "bgr")):
                    ps = pp.tile([128, F], dt.float32, tag="psA")
                    nc.tensor.matmul(ps[:], lhsT=lhs, rhs=wt[wn][:],
                                     start=True, stop=False)
                    nc.tensor.matmul(ps[:], lhsT=ones_sb[:], rhs=wt[bn_][:],
                                     start=False, stop=True)
                    ob = sbw.tile([128, F], dt.bfloat16, tag="stA")
                    nc.vector.tensor_copy(ob[:], ps[:])
                    rows = slice(128 * ci, 128 * (ci + 1))
                    if wn == "Wgl":
                        nc.sync.dma_start(xg_loc[rows, :], ob[:])
                    else:
                        nc.sync.dma_start(xr_loc[rows, 0:F], ob[:])
                        nc.sync.dma_start(xr_loc[rows, F:2 * F], ob[:])
            nc.gpsimd.collective_compute(
                "AllGather", alu.bypass, replica_groups=rg,
                ins=[xg_loc.opt()], outs=[xg_full.opt()])

            tab_pair = xg_full[:].rearrange("(a two) f -> a (two f)", two=2)

            def gather(dst_tile, tab, idx_sb, c):
                nc.gpsimd.dma_gather(
                    out_ap=dst_tile[:].rearrange("p (b f) -> p b f", f=2 * F),
                    in_ap=tab,
                    idxs_ap=idx_sb[:, (NI // 16) * c:(NI // 16) * (c + 1)],
                    num_idxs=NI, num_idxs_reg=NI, elem_size=2 * F,
                    single_packet=False)

            # ---- pass 1: logits ----
            W2 = 2 * F * KC
            for c in range(nchunks):
                sl = slice(c * KC, (c + 1) * KC)
                gp = sbw.tile([128, W2], dt.bfloat16, tag="gp", bufs=3)
                gx = sbw.tile([128, W2], dt.bfloat16, tag="gx", bufs=3)
                gather(gp, tab_pair, iP_sb, c)
                gather(gx, xr_loc[:], iX_sb, c)
                # stash the raw gathered pairs for pass 2
                nc.sync.dma_start(xls_scr[:, W2 * c:W2 * (c + 1)], gp[:])
                nc.vector.tensor_tensor(out=gx[:], in0=gx[:], in1=gp[:],
                                        op=alu.add)
                zab = sbw.tile([128, W2], dt.bfloat16, tag="zab", bufs=3)
                nc.scalar.activation(zab[:], gx[:], act.Abs, scale=0.4)
                nc.vector.scalar_tensor_tensor(
                    out=zab[:], in0=gx[:], scalar=0.6, in1=zab[:],
                    op0=alu.mult, op1=alu.add)
                m3 = zab[:].rearrange("p (k f) -> p k f", f=2 * F)
                lgp = sbw.tile([128, KC], dt.float32, tag="lgp")
                lgn = sbw.tile([128, KC], dt.float32, tag="lgn")
                lgp1 = sbw.tile([128, KC], dt.float32, tag="lgp1")
                lgn1 = sbw.tile([128, KC], dt.float32, tag="lgn1")
                nc.vector.tensor_reduce(lgp[:], m3[:, :, 0:PP],
                                        axis=mybir.AxisListType.X, op=alu.add)
                nc.vector.tensor_reduce(lgn[:], m3[:, :, PP:F],
                                        axis=mybir.AxisListType.X, op=alu.add)
                nc.vector.tensor_reduce(lgp1[:], m3[:, :, F:F + PP],
                                        axis=mybir.AxisListType.X, op=alu.add)
                nc.vector.tensor_reduce(lgn1[:], m3[:, :, F + PP:2 * F],
                                        axis=mybir.AxisListType.X, op=alu.add)
                # a = p0-n0 ; b = p1-n1 ; logit = a + par*(b-a)
                nc.vector.tensor_tensor(out=lgp[:], in0=lgp[:], in1=lgn[:],
                                        op=alu.subtract)
                nc.vector.tensor_tensor(out=lgp1[:], in0=lgp1[:], in1=lgn1[:],
                                        op=alu.subtract)
                nc.vector.tensor_tensor(out=lgp1[:], in0=lgp1[:], in1=lgp[:],
                                        op=alu.subtract)
                nc.vector.tensor_tensor(out=lgp1[:], in0=lgp1[:],
                                        in1=par_sb[:, sl], op=alu.mult)
                nc.vector.tensor_tensor(out=logits_sb[:, sl], in0=lgp[:],
                                        in1=lgp1[:], op=alu.add)

            # ---- segment softmax ----
            if stage < 2:
                osb0 = sbp.tile([1, NCLS], dt.float32)
                nc.vector.tensor_reduce(osb0[:], logits_sb[0:1, 0:NCLS * 20].rearrange("o (a b) -> o a b", a=NCLS), axis=mybir.AxisListType.X, op=alu.add)
                nc.sync.dma_start(out, osb0[:])
            if stage >= 2:
                nc.scalar.activation(E_sb[:], logits_sb[:], act.Exp)
                nc.vector.tensor_tensor_scan(
                    out=S_sb[:], data0=mf_sb[:], data1=E_sb[:], initial=0.0,
                    op0=alu.mult, op1=alu.add)
                nc.vector.tensor_tensor_scan(
                    out=D_sb[:, ::-1], data0=mr_sb[:, ::-1], data1=S_sb[:, ::-1],
                    initial=0.0, op0=alu.mult, op1=alu.max)
                nc.vector.reciprocal(D_sb[:], D_sb[:])
                nc.vector.tensor_tensor(out=w_sb[:], in0=E_sb[:], in1=D_sb[:],
                                        op=alu.mult)
                nc.vector.tensor_tensor(out=w_sb[:], in0=w_sb[:], in1=mv_sb[:],
                                        op=alu.mult)
                # w1 = w*par ; w0 = w - w1
                nc.vector.tensor_tensor(out=S_sb[:], in0=w_sb[:], in1=par_sb[:],
                                        op=alu.mult)
                nc.vector.tensor_tensor(out=D_sb[:], in0=w_sb[:], in1=S_sb[:],
                                        op=alu.subtract)
                nc.vector.tensor_copy(wb_sb[:], D_sb[:])      # w0 bf16
                nc.vector.tensor_copy(wb1_sb[:], S_sb[:])     # w1 bf16

            # ---- pass 2: partial = sum_slots w * xg[src] ----
            if stage == 2:
                osb1 = sbp.tile([1, NCLS], dt.float32)
                nc.vector.tensor_reduce(osb1[:], w_sb[0:1, 0:NCLS * 20].rearrange("o (a b) -> o a b", a=NCLS), axis=mybir.AxisListType.X, op=alu.add)
                nc.sync.dma_start(out, osb1[:])
            if stage >= 3:
                pacc = pp1.tile([F, 1], dt.float32, tag="pacc")
                first = True
                for c in range(nchunks):
                    gp = sbw.tile([128, W2], dt.bfloat16, tag="gp", bufs=3)
                    nc.sync.dma_start(gp[:], xls_scr[:, W2 * c:W2 * (c + 1)])
                    for b in range(KC):
                        col = slice(c * KC + b, c * KC + b + 1)
                        nc.tensor.matmul(
                            pacc[:], lhsT=gp[:, 2 * F * b:2 * F * b + F],
                            rhs=wb_sb[:, col], start=first, stop=False)
                        first = False
                        nc.tensor.matmul(
                            pacc[:], lhsT=gp[:, 2 * F * b + F:2 * F * (b + 1)],
                            rhs=wb1_sb[:, col], start=False,
                            stop=(c == nchunks - 1 and b == KC - 1))

                part_sb = sbp.tile([F, 1], dt.float32)
                nc.vector.tensor_copy(part_sb[:], pacc[:])
                # store the [F] partial as a flat row in DRAM
                nc.sync.dma_start(part_loc[:].rearrange("o f -> f o"), part_sb[:])

                nc.gpsimd.collective_compute(
                    "AllReduce", alu.add, replica_groups=rg,
                    ins=[part_loc.opt()], outs=[pooled.opt()])

                # ---- head ----
                pool_sb = sbp.tile([F, 1], dt.float32)
                nc.sync.dma_start(pool_sb[:], pooled[:].rearrange("o f -> f o"))
                Ap_sb = sbp.tile([F, 1], dt.float32)
                nc.sync.dma_start(Ap_sb[:], Ap)
                Bp_sb = sbp.tile([F, 1], dt.float32)
                nc.sync.dma_start(Bp_sb[:], Bp)
                Wc_sb = sbp.tile([F, NCLS], dt.float32)
                nc.sync.dma_start(Wc_sb[:], Wcp)
                bc_sb = sbp.tile([1, NCLS], dt.float32)
                nc.sync.dma_start(bc_sb[:], bc)
                h_sb = sbp.tile([F, 1], dt.float32)
                nc.vector.scalar_tensor_tensor(
                    out=h_sb[:], in0=pool_sb[:], scalar=Ap_sb[:, 0:1], in1=Bp_sb[:],
                    op0=alu.mult, op1=alu.add)
                one1 = sbp.tile([1, 1], dt.float32)
                nc.vector.memset(one1[:], 1.0)
                hp = pp1.tile([1, NCLS], dt.float32, tag="hp")
                nc.tensor.matmul(hp[:], lhsT=h_sb[:], rhs=Wc_sb[:], start=True,
                                 stop=False)
                nc.tensor.matmul(hp[:], lhsT=one1[:], rhs=bc_sb[:], start=False,
                                 stop=True)
                eh = sbp.tile([1, NCLS], dt.float32)
                nc.scalar.activation(eh[:], hp[:], act.Exp)
                den = sbp.tile([1, 1], dt.float32)
                nc.vector.tensor_reduce(den[:], eh[:], axis=mybir.AxisListType.X,
                                        op=alu.add)
                rden = sbp.tile([1, 1], dt.float32)
                nc.vector.reciprocal(rden[:], den[:])
                osb = sbp.tile([1, NCLS], dt.float32)
                nc.vector.tensor_scalar(out=osb[:], in0=eh[:], scalar1=rden[:, 0:1],
                                        scalar2=None, op0=alu.mult)
                nc.sync.dma_start(out, osb[:])

    nc.compile()
    return nc


# --------------------------------------------------------------------------
# public entry point
# --------------------------------------------------------------------------

_CACHE = {}


def _install_ntff_hook():
    """Provide antenv.axon_hooks + the ctypes NTFF hook when the image lacks
    them, so run_bass_kernel_spmd(trace=True) can capture exec_time_ns."""
    import contextlib
    import ctypes
    import sys
    import types

    try:
        import antenv.axon_hooks  # noqa: F401
        return
    except ImportError:
        pass
    try:
        import antenv
    except ImportError:
        return
    holder = [None]
    mod = types.ModuleType("antenv.axon_hooks")
    mod.set_axon_ntff_profile_hook = lambda h: holder.__setitem__(0, h)
    mod.get_axon_ntff_profile_hook = lambda: holder[0]
    sys.modules["antenv.axon_hooks"] = mod
    antenv.axon_hooks = mod

    so_path = "/opt/axon/libaxon_pjrt.so"
    if os.path.exists(so_path):
        lib = ctypes.CDLL(so_path)
        if hasattr(lib, "axon_start_nrt_profile"):
            lib.axon_start_nrt_profile.argtypes = [
                ctypes.POINTER(ctypes.c_int64), ctypes.c_size_t]
            lib.axon_start_nrt_profile.restype = ctypes.c_int64
            lib.axon_stop_nrt_profile.argtypes = [ctypes.c_char_p]
            lib.axon_stop_nrt_profile.restype = ctypes.c_int64

            @contextlib.contextmanager
            def _hook(output_dir, device_ids):
                import jax
                jax.devices()
                if device_ids:
                    ids = (ctypes.c_int64 * len(device_ids))(*device_ids)
                    rc = lib.axon_start_nrt_profile(ids, len(device_ids))
                else:
                    rc = lib.axon_start_nrt_profile(None, 0)
                if rc != 0:
                    raise RuntimeError(f"axon_start_nrt_profile rc={rc}")
                try:
                    yield
                finally:
                    n = lib.axon_stop_nrt_profile(str(output_dir).encode())
                    print(f"ntff profile: {n} file(s) -> {output_dir}")

            mod.set_axon_ntff_profile_hook(_hook)

    import concourse.bass_utils as bu
    bu.upload_artifacts = lambda tmpdir: "local://" + str(tmpdir)


def kernel(**inputs):
    from concourse.bass_utils import run_bass_kernel_spmd

    if bool(int(os.environ.get("KERNEL_TRACE", "0"))):
        _install_ntff_hook()
    inputs = {k: np.asarray(v) for k, v in inputs.items()}
    in_maps, meta = prep_host(**inputs)
    key = tuple(sorted(meta.items()))
    if key not in _CACHE:
        _CACHE[key] = build(meta)
    nc = _CACHE[key]
    res = run_bass_kernel_spmd(nc, in_maps, core_ids=list(range(M)),
                               trace=bool(int(os.environ.get("KERNEL_TRACE", "0"))))
    if getattr(res, "exec_time_ns", None) is not None:
        print(f"HW exec time: {res.exec_time_ns} ns")
    return np.asarray(res.results[0]["out"]).astype(np.float32)



# revision 6
# speedup vs baseline: 1.5444x; 1.5444x over previous
"""Distributed Trainium2 kernel for a GATv2 layer + BN + global-mean-pool + classifier.

Math (reference, heads=1):
    xl = x@Wl + bl ; xr = x@Wr + br
    logit_e = att . leaky_relu(xl[src_e] + xr[dst_e], 0.2)
    a_e     = segment_softmax(logit_e over dst)
    out_i   = sum_{e: dst=i} a_e * xl[src_e] ; out = out + bias1
    h       = BN(out) ; g = mean_i h ; y = softmax(g@Wc + bc)

The output is a global mean over nodes and BN is affine per feature, so
per-node outputs never materialize:
    y = softmax( ((S/N)*A + B) @ Wc + bc ),  S = sum_e a_e * xl[src_e],
    A = gamma/sqrt(var+eps), B = (bias1 - mu)*A + beta.

Attention weights v = att are folded into the gather tables host-side:
    v_f * lrelu(z_f) = sign_f * lrelu(|v_f| z_f)
with features permuted so positive-sign features occupy columns [0,PP).

Distribution over 8 cores: nodes sharded contiguously; edges sharded by dst.
Per core the edges are packed, whole dst-segments at a time, into
(chunk, partition) bins of KC slots — no segment straddles a chunk — and
each segment is padded to a multiple of 4 slots.  Per chunk:
  * xl[src] pair-rows come from one bulk dma_gather (int16 idx, 512B elems)
  * xr[dst] comes from a QUAD dma_gather (2048B elems = 4 slots' worth of
    duplicated xr rows -> 4x fewer descriptors; SWDGE cost is per descriptor)
  * logits: z = gl+gr ; lrelu(z) = max(z, 0.2z) in place; 4 masked reduces
  * segment softmax via chunk-local masked fwd add-scan + rev max-scan
  * weighted sum: per-slot [128,F]^T @ w matmuls accumulate into PSUM while
    the gathered rows are still in SBUF (single pass, no DRAM stash).
AllReduce of the [F] partial + a tiny head finishes.
"""

import math
import os

import ml_dtypes
import numpy as np

import concourse.bass as bass
import concourse.bacc as bacc
import concourse.mybir as mybir
import concourse.tile as tile

M = 8  # cores
F = 128
NCLS = 5
BN_EPS = 1e-5
KC = 48  # slot-columns per chunk

BF16 = ml_dtypes.bfloat16


def _wrap_idx(seq):
    """[N] int array -> [128, N//16] int16 wrap layout (16-partition groups,
    replicated across the 8 gpsimd cores)."""
    n = seq.shape[0]
    assert n % 16 == 0
    w = seq.reshape(n // 16, 16).T.astype(np.int16)
    return np.tile(w, (8, 1))


def _pack_bins(qlen, kc, nbase):
    """Pack segments (sizes qlen, each <= kc) whole into 128*nch bins of
    capacity kc.  Returns (nch, bin_of_seg, off_of_seg) with bins balanced
    (always place into least-loaded bin that fits)."""
    order = np.argsort(-qlen, kind="stable")
    nch = nbase
    while True:
        nbins = 128 * nch
        loads = np.zeros(nbins, np.int64)
        binof = np.zeros(qlen.shape[0], np.int64)
        offof = np.zeros(qlen.shape[0], np.int64)
        ok = True
        for g in order:
            q = qlen[g]
            b = int(np.argmin(loads))
            if loads[b] + q > kc:
                # least-loaded doesn't fit (only when nearly full): find any fit
                cand = np.nonzero(loads + q <= kc)[0]
                if cand.size == 0:
                    ok = False
                    break
                b = int(cand[np.argmin(loads[cand])])
            binof[g] = b
            offof[g] = loads[b]
            loads[b] += q
        if ok:
            return nch, binof, offof
        nch += 1


def prep_host(x, edge_index, Wl, bl, Wr, br, att, bias1,
              bn_gamma, bn_beta, bn_mean, bn_var, Wc, bc):
    N = x.shape[0]
    npc = N // M
    assert npc * M == N
    NPC = ((npc + 1 + 127) // 128) * 128  # always >= 1 pad row (dummy)
    CH = NPC // 128
    NG = M * NPC
    DUM = npc  # first pad row of each core's shard (zeros + bias content)

    src = np.concatenate([edge_index[0], np.arange(N, dtype=np.int64)])
    dst = np.concatenate([edge_index[1], np.arange(N, dtype=np.int64)])

    # ---- attention folding ----
    v = np.asarray(att[0], np.float64)
    posm = v >= 0
    perm = np.argsort(~posm, kind="stable")
    PP = int(posm.sum())
    assert 0 < PP < F, f"degenerate attention sign split PP={PP}"
    absv = np.abs(v[perm])
    Wg_l = (Wl[:, perm] * absv[None, :]).astype(np.float32)
    bg_l = (bl[perm] * absv).astype(np.float32)
    Wg_r = (Wr[:, perm] * absv[None, :]).astype(np.float32)
    bg_r = (br[perm] * absv).astype(np.float32)

    # ---- per-core grid packing (whole segments, quad-padded, chunk-local) ----
    deg = np.bincount(dst, minlength=N)
    assert deg.min() >= 1
    qlen_all = ((deg + 3) // 4) * 4
    assert qlen_all.max() <= KC, f"segment of {qlen_all.max()} slots > KC={KC}"

    nch = 0
    binof = np.zeros(N, np.int64)
    offof = np.zeros(N, np.int64)
    for k in range(M):
        ql = qlen_all[k * npc:(k + 1) * npc]
        nbase = (int(ql.sum()) + 128 * KC - 1) // (128 * KC)
        nck, bk, ok_ = _pack_bins(ql, KC, nbase)
        nch = max(nch, nck)
        binof[k * npc:(k + 1) * npc] = bk
        offof[k * npc:(k + 1) * npc] = ok_
    L = nch * KC
    LQ = L // 4

    # per-edge slot position: sort edges by dst, enumerate within segment
    order = np.argsort(dst, kind="stable")
    ds = dst[order]
    ss = src[order]
    n_e = ds.shape[0]
    start = np.ones(n_e, bool)
    start[1:] = ds[1:] != ds[:-1]
    idxs = np.arange(n_e, dtype=np.int64)
    first = np.maximum.accumulate(np.where(start, idxs, 0))
    q = idxs - first                      # rank within segment

    seg_bin = binof[ds]                   # bin = chunk*128 + partition
    seg_c = seg_bin // 128
    seg_p = seg_bin % 128
    t = seg_c * KC + offof[ds] + q        # global slot column
    pd = seg_p
    cd = ds // npc

    srcrow = (ss // npc) * NPC + (ss % npc)

    iP = np.full((M, 128, L), 0, np.int64)
    par = np.zeros((M, 128, L), np.float32)
    mask_f = np.zeros((M, 128, L), np.float32)
    mask_r = np.zeros((M, 128, L), np.float32)
    mask_v = np.zeros((M, 128, L), np.float32)
    d_end = np.ones(n_e, bool)
    d_end[:-1] = start[1:]

    iP[cd, pd, t] = srcrow >> 1
    par[cd, pd, t] = (srcrow & 1).astype(np.float32)
    mask_f[cd, pd, t] = (~start).astype(np.float32)
    mask_r[cd, pd, t] = (~d_end).astype(np.float32)
    mask_v[cd, pd, t] = 1.0
    # pad slots keep mask_f = mask_r = mask_v = 0 and iP -> own core's DUM pair
    for k in range(M):
        pad = mask_v[k] == 0.0
        iP[k][pad] = (k * NPC + DUM) >> 1

    # quad xr index: quad (p, qd) -> local dst row of the segment covering it
    iXq = np.full((M, 128, LQ), DUM, np.int64)
    dls = ds - cd * npc
    # every quad within a segment has a real edge at its first slot
    sel = (q % 4) == 0
    iXq[cd[sel], pd[sel], (t[sel] // 4)] = dls[sel]

    def to_wrap(a):  # [128, X] -> wrap over i = col*128 + p sequence
        seq = a.T.reshape(-1)
        return _wrap_idx(seq)

    iP_w = np.stack([to_wrap(iP[k]) for k in range(M)])
    iX_w = np.stack([to_wrap(iXq[k]) for k in range(M)])

    # ---- head constants (de-permuted / de-scaled) ----
    A = bn_gamma.astype(np.float64) / np.sqrt(bn_var.astype(np.float64) + BN_EPS)
    Ap = (A[perm] / (N * absv)).astype(np.float32).reshape(F, 1)
    Bp = ((bias1 - bn_mean).astype(np.float64) * A + bn_beta)[perm] \
        .astype(np.float32).reshape(F, 1)
    Wcp = Wc[perm, :].astype(np.float32)

    # ---- per-core x^T (padded, bf16) ----
    xT = np.zeros((M, 128, NPC), BF16)
    for k in range(M):
        xT[k, :, :npc] = x[k * npc:(k + 1) * npc].T.astype(BF16)

    meta = dict(NPC=NPC, CH=CH, NG=NG, L=L, PP=PP, N=N, DUM=DUM, NCH=nch)

    in_maps = []
    for k in range(M):
        in_maps.append({
            "xT": np.ascontiguousarray(xT[k]),
            "Wgl": Wg_l.astype(BF16),
            "bgl": bg_l.reshape(1, F).astype(BF16),
            "Wgr": Wg_r.astype(BF16),
            "bgr": bg_r.reshape(1, F).astype(BF16),
            "iP": np.ascontiguousarray(iP_w[k]),
            "iX": np.ascontiguousarray(iX_w[k]),
            "par": np.ascontiguousarray(par[k]),
            "mask_f": np.ascontiguousarray(mask_f[k]),
            "mask_r": np.ascontiguousarray(mask_r[k]),
            "mask_v": np.ascontiguousarray(mask_v[k]),
            "Ap": Ap,
            "Bp": Bp,
            "Wcp": Wcp,
            "bc": bc.reshape(1, NCLS).astype(np.float32),
        })
    return in_maps, meta


def build(meta):
    NPC, CH, NG, L, PP, DUM, NCH = (
        meta[k] for k in ("NPC", "CH", "NG", "L", "PP", "DUM", "NCH"))
    LQ = L // 4
    NI = KC * 128           # pair-gather indices per chunk
    NIQ = (KC // 4) * 128   # quad-gather indices per chunk
    LW = (L * 128) // 16
    LWQ = (LQ * 128) // 16
    W2 = 2 * F * KC

    dt = mybir.dt
    alu = mybir.AluOpType
    act = mybir.ActivationFunctionType
    rg = [list(range(M))]

    nc = bacc.Bacc("TRN2", target_bir_lowering=False, debug=False, num_devices=M)

    def p_in(name, shape, d):
        return nc.dram_tensor(name, shape, d, kind="ExternalInput").ap()

    xT = p_in("xT", [128, NPC], dt.bfloat16)
    Wgl = p_in("Wgl", [F, F], dt.bfloat16)
    bgl = p_in("bgl", [1, F], dt.bfloat16)
    Wgr = p_in("Wgr", [F, F], dt.bfloat16)
    bgr = p_in("bgr", [1, F], dt.bfloat16)
    iP = p_in("iP", [128, LW], dt.int16)
    iX = p_in("iX", [128, LWQ], dt.int16)
    par = p_in("par", [128, L], dt.float32)
    mask_f = p_in("mask_f", [128, L], dt.float32)
    mask_r = p_in("mask_r", [128, L], dt.float32)
    mask_v = p_in("mask_v", [128, L], dt.float32)
    Ap = p_in("Ap", [F, 1], dt.float32)
    Bp = p_in("Bp", [F, 1], dt.float32)
    Wcp = p_in("Wcp", [F, NCLS], dt.float32)
    bc = p_in("bc", [1, NCLS], dt.float32)
    out = nc.dram_tensor("out", [1, NCLS], dt.float32, kind="ExternalOutput").ap()

    with tile.TileContext(nc) as tc:
        with (
            tc.tile_pool(name="dram", bufs=1, space="DRAM") as dpool,
            tc.tile_pool(name="sbp", bufs=1) as sbp,
            tc.tile_pool(name="sbw", bufs=2) as sbw,
            tc.tile_pool(name="ps2", bufs=2, space="PSUM") as pp,
            tc.tile_pool(name="ps1", bufs=1, space="PSUM") as pp1,
        ):
            xg_loc = dpool.tile([NPC, F], dt.bfloat16)
            xr_mini = dpool.tile([NPC, F], dt.bfloat16)
            xrq = dpool.tile([NPC, 8 * F], dt.bfloat16)
            xg_full = dpool.tile([NG, F], dt.bfloat16, addr_space="Shared")
            part_loc = dpool.tile([1, F], dt.float32)
            pooled = dpool.tile([1, F], dt.float32, addr_space="Shared")

            # ---- persistent SBUF ----
            xT_sb = sbp.tile([128, NPC], dt.bfloat16)
            nc.sync.dma_start(xT_sb[:], xT)
            wt = {}
            for nm, apin, sh in (("Wgl", Wgl, [F, F]), ("bgl", bgl, [1, F]),
                                 ("Wgr", Wgr, [F, F]), ("bgr", bgr, [1, F])):
                tl = sbp.tile(sh, dt.bfloat16, tag=nm)
                nc.sync.dma_start(tl[:], apin)
                wt[nm] = tl
            ones_sb = sbp.tile([1, F], dt.bfloat16)
            nc.vector.memset(ones_sb[:], 1.0)

            iP_sb = sbp.tile([128, LW], dt.int16)
            nc.sync.dma_start(iP_sb[:], iP)
            iX_sb = sbp.tile([128, LWQ], dt.int16)
            nc.sync.dma_start(iX_sb[:], iX)
            par_sb = sbp.tile([128, L], dt.float32)
            nc.sync.dma_start(par_sb[:], par)
            mf_sb = sbp.tile([128, L], dt.float32)
            nc.sync.dma_start(mf_sb[:], mask_f)
            mr_sb = sbp.tile([128, L], dt.float32)
            nc.sync.dma_start(mr_sb[:], mask_r)
            mv_sb = sbp.tile([128, L], dt.float32)
            nc.sync.dma_start(mv_sb[:], mask_v)

            # ---- stage A: node tables (xr first so quad gathers start early) ----
            for ci in range(CH):
                lhs = xT_sb[:, 128 * ci:128 * (ci + 1)]
                ps = pp.tile([128, F], dt.float32, tag="psA")
                nc.tensor.matmul(ps[:], lhsT=lhs, rhs=wt["Wgr"][:],
                                 start=True, stop=False)
                nc.tensor.matmul(ps[:], lhsT=ones_sb[:], rhs=wt["bgr"][:],
                                 start=False, stop=True)
                ob = sbw.tile([128, F], dt.bfloat16, tag="stA")
                nc.vector.tensor_copy(ob[:], ps[:])
                nc.sync.dma_start(xr_mini[128 * ci:128 * (ci + 1), :], ob[:])
            # duplicate xr rows 8x: quad table row j = [xr_j]*8
            for i in range(8):
                nc.sync.dma_start(
                    xrq[:].rearrange("a (e f) -> a e f", e=8)[:, i, :], xr_mini[:])

            for ci in range(CH):
                lhs = xT_sb[:, 128 * ci:128 * (ci + 1)]
                ps = pp.tile([128, F], dt.float32, tag="psA")
                nc.tensor.matmul(ps[:], lhsT=lhs, rhs=wt["Wgl"][:],
                                 start=True, stop=False)
                nc.tensor.matmul(ps[:], lhsT=ones_sb[:], rhs=wt["bgl"][:],
                                 start=False, stop=True)
                ob = sbw.tile([128, F], dt.bfloat16, tag="stA")
                nc.vector.tensor_copy(ob[:], ps[:])
                nc.sync.dma_start(xg_loc[128 * ci:128 * (ci + 1), :], ob[:])
            nc.gpsimd.collective_compute(
                "AllGather", alu.bypass, replica_groups=rg,
                ins=[xg_loc.opt()], outs=[xg_full.opt()])

            tab_pair = xg_full[:].rearrange("(a two) f -> a (two f)", two=2)

            # ---- fused pass over chunks ----
            def gather_pair(dst_tile, c):
                nc.gpsimd.dma_gather(
                    out_ap=dst_tile[:].rearrange("p (b f) -> p b f", f=2 * F),
                    in_ap=tab_pair,
                    idxs_ap=iP_sb[:, (NI // 16) * c:(NI // 16) * (c + 1)],
                    num_idxs=NI, num_idxs_reg=NI, elem_size=2 * F,
                    single_packet=False)

            def gather_quad(dst_tile, c):
                nc.gpsimd.dma_gather(
                    out_ap=dst_tile[:].rearrange("p (b f) -> p b f", f=8 * F),
                    in_ap=xrq[:],
                    idxs_ap=iX_sb[:, (NIQ // 16) * c:(NIQ // 16) * (c + 1)],
                    num_idxs=NIQ, num_idxs_reg=NIQ, elem_size=8 * F,
                    single_packet=False)

            LEAD = min(2, NCH)
            gx_tiles = {}
            for c in range(LEAD):
                gx = sbw.tile([128, W2], dt.bfloat16, tag="gx", bufs=3)
                gather_quad(gx, c)
                gx_tiles[c] = gx

            pacc = pp1.tile([F, 1], dt.float32, tag="pacc")
            for c in range(NCH):
                sl = slice(c * KC, (c + 1) * KC)
                gp = sbw.tile([128, W2], dt.bfloat16, tag="gp", bufs=3)
                gather_pair(gp, c)
                if c + LEAD < NCH:
                    gxn = sbw.tile([128, W2], dt.bfloat16, tag="gx", bufs=3)
                    gather_quad(gxn, c + LEAD)
                    gx_tiles[c + LEAD] = gxn
                gx = gx_tiles.pop(c)

                # z = gl + gr ; lrelu(z) = max(z, 0.2 z)   (all in place)
                nc.vector.tensor_tensor(out=gx[:], in0=gx[:], in1=gp[:],
                                        op=alu.add)
                nc.vector.scalar_tensor_tensor(
                    out=gx[:], in0=gx[:], scalar=0.2, in1=gx[:],
                    op0=alu.mult, op1=alu.max)
                m3 = gx[:].rearrange("p (k f) -> p k f", f=2 * F)
                lgp = sbw.tile([128, KC], dt.float32, tag="lgp")
                lgn = sbw.tile([128, KC], dt.float32, tag="lgn")
                lgp1 = sbw.tile([128, KC], dt.float32, tag="lgp1")
                lgn1 = sbw.tile([128, KC], dt.float32, tag="lgn1")
                nc.vector.tensor_reduce(lgp[:], m3[:, :, 0:PP],
                                        axis=mybir.AxisListType.X, op=alu.add)
                nc.vector.tensor_reduce(lgn[:], m3[:, :, PP:F],
                                        axis=mybir.AxisListType.X, op=alu.add)
                nc.vector.tensor_reduce(lgp1[:], m3[:, :, F:F + PP],
                                        axis=mybir.AxisListType.X, op=alu.add)
                nc.vector.tensor_reduce(lgn1[:], m3[:, :, F + PP:2 * F],
                                        axis=mybir.AxisListType.X, op=alu.add)
                # a = p0-n0 ; b = p1-n1 ; logit = a + par*(b-a)
                nc.vector.tensor_tensor(out=lgp[:], in0=lgp[:], in1=lgn[:],
                                        op=alu.subtract)
                nc.vector.tensor_tensor(out=lgp1[:], in0=lgp1[:], in1=lgn1[:],
                                        op=alu.subtract)
                nc.vector.tensor_tensor(out=lgp1[:], in0=lgp1[:], in1=lgp[:],
                                        op=alu.subtract)
                nc.vector.tensor_tensor(out=lgp1[:], in0=lgp1[:],
                                        in1=par_sb[:, sl], op=alu.mult)
                nc.vector.tensor_tensor(out=lgn[:], in0=lgp[:],
                                        in1=lgp1[:], op=alu.add)

                # chunk-local segment softmax
                E = sbw.tile([128, KC], dt.float32, tag="E")
                S = sbw.tile([128, KC], dt.float32, tag="S")
                D = sbw.tile([128, KC], dt.float32, tag="D")
                nc.scalar.activation(E[:], lgn[:], act.Exp)
                nc.vector.tensor_tensor_scan(
                    out=S[:], data0=mf_sb[:, sl], data1=E[:], initial=0.0,
                    op0=alu.mult, op1=alu.add)
                nc.vector.tensor_tensor_scan(
                    out=D[:, ::-1], data0=mr_sb[:, sl][:, ::-1],
                    data1=S[:, ::-1], initial=0.0, op0=alu.mult, op1=alu.max)
                nc.vector.reciprocal(D[:], D[:])
                nc.vector.tensor_tensor(out=E[:], in0=E[:], in1=D[:],
                                        op=alu.mult)
                nc.vector.tensor_tensor(out=E[:], in0=E[:], in1=mv_sb[:, sl],
                                        op=alu.mult)
                # w1 = w*par ; w0 = w - w1
                nc.vector.tensor_tensor(out=S[:], in0=E[:], in1=par_sb[:, sl],
                                        op=alu.mult)
                nc.vector.tensor_tensor(out=D[:], in0=E[:], in1=S[:],
                                        op=alu.subtract)
                wb0 = sbw.tile([128, KC], dt.bfloat16, tag="wb0")
                wb1 = sbw.tile([128, KC], dt.bfloat16, tag="wb1")
                nc.vector.tensor_copy(wb0[:], D[:])
                nc.vector.tensor_copy(wb1[:], S[:])

                # weighted sum while gp is in SBUF
                for b in range(KC):
                    nc.tensor.matmul(
                        pacc[:], lhsT=gp[:, 2 * F * b:2 * F * b + F],
                        rhs=wb0[:, b:b + 1], start=(c == 0 and b == 0),
                        stop=False)
                    nc.tensor.matmul(
                        pacc[:], lhsT=gp[:, 2 * F * b + F:2 * F * (b + 1)],
                        rhs=wb1[:, b:b + 1], start=False,
                        stop=(c == NCH - 1 and b == KC - 1))

            part_sb = sbp.tile([F, 1], dt.float32)
            nc.vector.tensor_copy(part_sb[:], pacc[:])
            nc.sync.dma_start(part_loc[:].rearrange("o f -> f o"), part_sb[:])

            nc.gpsimd.collective_compute(
                "AllReduce", alu.add, replica_groups=rg,
                ins=[part_loc.opt()], outs=[pooled.opt()])

            # ---- head ----
            pool_sb = sbp.tile([F, 1], dt.float32)
            nc.sync.dma_start(pool_sb[:], pooled[:].rearrange("o f -> f o"))
            Ap_sb = sbp.tile([F, 1], dt.float32)
            nc.sync.dma_start(Ap_sb[:], Ap)
            Bp_sb = sbp.tile([F, 1], dt.float32)
            nc.sync.dma_start(Bp_sb[:], Bp)
            Wc_sb = sbp.tile([F, NCLS], dt.float32)
            nc.sync.dma_start(Wc_sb[:], Wcp)
            bc_sb = sbp.tile([1, NCLS], dt.float32)
            nc.sync.dma_start(bc_sb[:], bc)
            h_sb = sbp.tile([F, 1], dt.float32)
            nc.vector.scalar_tensor_tensor(
                out=h_sb[:], in0=pool_sb[:], scalar=Ap_sb[:, 0:1], in1=Bp_sb[:],
                op0=alu.mult, op1=alu.add)
            one1 = sbp.tile([1, 1], dt.float32)
            nc.vector.memset(one1[:], 1.0)
            hp = pp1.tile([1, NCLS], dt.float32, tag="hp")
            nc.tensor.matmul(hp[:], lhsT=h_sb[:], rhs=Wc_sb[:], start=True,
                             stop=False)
            nc.tensor.matmul(hp[:], lhsT=one1[:], rhs=bc_sb[:], start=False,
                             stop=True)
            eh = sbp.tile([1, NCLS], dt.float32)
            nc.scalar.activation(eh[:], hp[:], act.Exp)
            den = sbp.tile([1, 1], dt.float32)
            nc.vector.tensor_reduce(den[:], eh[:], axis=mybir.AxisListType.X,
                                    op=alu.add)
            rden = sbp.tile([1, 1], dt.float32)
            nc.vector.reciprocal(rden[:], den[:])
            osb = sbp.tile([1, NCLS], dt.float32)
            nc.vector.tensor_scalar(out=osb[:], in0=eh[:], scalar1=rden[:, 0:1],
                                    scalar2=None, op0=alu.mult)
            nc.sync.dma_start(out, osb[:])

    nc.compile()
    return nc


# --------------------------------------------------------------------------
# public entry point
# --------------------------------------------------------------------------

_CACHE = {}


def _install_ntff_hook():
    """Provide antenv.axon_hooks + the ctypes NTFF hook when the image lacks
    them, so run_bass_kernel_spmd(trace=True) can capture exec_time_ns."""
    import contextlib
    import ctypes
    import sys
    import types

    try:
        import antenv.axon_hooks  # noqa: F401
        return
    except ImportError:
        pass
    try:
        import antenv
    except ImportError:
        return
    holder = [None]
    mod = types.ModuleType("antenv.axon_hooks")
    mod.set_axon_ntff_profile_hook = lambda h: holder.__setitem__(0, h)
    mod.get_axon_ntff_profile_hook = lambda: holder[0]
    sys.modules["antenv.axon_hooks"] = mod
    antenv.axon_hooks = mod

    so_path = "/opt/axon/libaxon_pjrt.so"
    if os.path.exists(so_path):
        lib = ctypes.CDLL(so_path)
        if hasattr(lib, "axon_start_nrt_profile"):
            lib.axon_start_nrt_profile.argtypes = [
                ctypes.POINTER(ctypes.c_int64), ctypes.c_size_t]
            lib.axon_start_nrt_profile.restype = ctypes.c_int64
            lib.axon_stop_nrt_profile.argtypes = [ctypes.c_char_p]
            lib.axon_stop_nrt_profile.restype = ctypes.c_int64

            @contextlib.contextmanager
            def _hook(output_dir, device_ids):
                import jax
                jax.devices()
                if device_ids:
                    ids = (ctypes.c_int64 * len(device_ids))(*device_ids)
                    rc = lib.axon_start_nrt_profile(ids, len(device_ids))
                else:
                    rc = lib.axon_start_nrt_profile(None, 0)
                if rc != 0:
                    raise RuntimeError(f"axon_start_nrt_profile rc={rc}")
                try:
                    yield
                finally:
                    n = lib.axon_stop_nrt_profile(str(output_dir).encode())
                    print(f"ntff profile: {n} file(s) -> {output_dir}")

            mod.set_axon_ntff_profile_hook(_hook)

    import concourse.bass_utils as bu
    bu.upload_artifacts = lambda tmpdir: "local://" + str(tmpdir)


def kernel(**inputs):
    from concourse.bass_utils import run_bass_kernel_spmd

    if bool(int(os.environ.get("KERNEL_TRACE", "0"))):
        _install_ntff_hook()
    inputs = {k: np.asarray(v) for k, v in inputs.items()}
    in_maps, meta = prep_host(**inputs)
    key = tuple(sorted(meta.items()))
    if key not in _CACHE:
        _CACHE[key] = build(meta)
    nc = _CACHE[key]
    res = run_bass_kernel_spmd(nc, in_maps, core_ids=list(range(M)),
                               trace=bool(int(os.environ.get("KERNEL_TRACE", "0"))))
    if getattr(res, "exec_time_ns", None) is not None:
        print(f"HW exec time: {res.exec_time_ns} ns")
    return np.asarray(res.results[0]["out"]).astype(np.float32)


# revision 8
# speedup vs baseline: 1.5519x; 1.0049x over previous
"""Distributed Trainium2 kernel for a GATv2 layer + BN + global-mean-pool + classifier.

Math (reference, heads=1):
    xl = x@Wl + bl ; xr = x@Wr + br
    logit_e = att . leaky_relu(xl[src_e] + xr[dst_e], 0.2)
    a_e     = segment_softmax(logit_e over dst)
    out_i   = sum_{e: dst=i} a_e * xl[src_e] ; out = out + bias1
    h       = BN(out) ; g = mean_i h ; y = softmax(g@Wc + bc)

The output is a global mean over nodes and BN is affine per feature, so
per-node outputs never materialize:
    y = softmax( ((S/N)*A + B) @ Wc + bc ),  S = sum_e a_e * xl[src_e],
    A = gamma/sqrt(var+eps), B = (bias1 - mu)*A + beta.

Attention weights v = att are folded into the gather tables host-side:
    v_f * lrelu(z_f) = sign_f * lrelu(|v_f| z_f)
with features permuted so positive-sign features occupy columns [0,PP).

Distribution over 8 cores: nodes sharded contiguously; edges sharded by dst.
Per core the edges are packed, whole dst-segments at a time, into
(chunk, partition) bins of KC slots — no segment straddles a chunk — and
each segment is padded to a multiple of 4 slots.  Per chunk:
  * xl[src] pair-rows come from one bulk dma_gather (int16 idx, 512B elems)
  * xr[dst] comes from a QUAD dma_gather (2048B elems = 4 slots' worth of
    duplicated xr rows -> 4x fewer descriptors; SWDGE cost is per descriptor)
  * logits: z = gl+gr ; lrelu(z) = max(z, 0.2z) in place; 4 masked reduces
  * segment softmax via chunk-local masked fwd add-scan + rev max-scan
  * weighted sum: per-slot [128,F]^T @ w matmuls accumulate into PSUM while
    the gathered rows are still in SBUF (single pass, no DRAM stash).
AllReduce of the [F] partial + a tiny head finishes.
"""

import math
import os

import ml_dtypes
import numpy as np

import concourse.bass as bass
import concourse.bacc as bacc
import concourse.mybir as mybir
import concourse.tile as tile

M = 8  # cores
F = 128
NCLS = 5
BN_EPS = 1e-5
KC = 48  # slot-columns per chunk

BF16 = ml_dtypes.bfloat16


def _wrap_idx(seq):
    """[N] int array -> [128, N//16] int16 wrap layout (16-partition groups,
    replicated across the 8 gpsimd cores)."""
    n = seq.shape[0]
    assert n % 16 == 0
    w = seq.reshape(n // 16, 16).T.astype(np.int16)
    return np.tile(w, (8, 1))


def _pack_bins(qlen, kc, nbase):
    """Pack segments (sizes qlen, each <= kc) whole into 128*nch bins of
    capacity kc.  First-fit-decreasing, preferring earlier chunks (and within
    a chunk, fuller bins) so the final chunk stays light — it sets the
    pipeline tail.  Returns (nch, bin_of_seg, off_of_seg); bin = chunk*128+p."""
    order = np.argsort(-qlen, kind="stable")
    nch = nbase
    while True:
        nbins = 128 * nch
        loads = np.zeros(nbins, np.int64)
        chunk_of = np.arange(nbins) // 128
        binof = np.zeros(qlen.shape[0], np.int64)
        offof = np.zeros(qlen.shape[0], np.int64)
        ok = True
        for g in order:
            q = qlen[g]
            cand = np.nonzero(loads + q <= kc)[0]
            if cand.size == 0:
                ok = False
                break
            # earliest chunk; within it the fullest bin
            key = chunk_of[cand] * (kc + 1) - loads[cand]
            b = int(cand[np.argmin(key)])
            binof[g] = b
            offof[g] = loads[b]
            loads[b] += q
        if ok:
            return nch, binof, offof
        nch += 1


def prep_host(x, edge_index, Wl, bl, Wr, br, att, bias1,
              bn_gamma, bn_beta, bn_mean, bn_var, Wc, bc):
    N = x.shape[0]
    npc = N // M
    assert npc * M == N
    NPC = ((npc + 1 + 127) // 128) * 128  # always >= 1 pad row (dummy)
    CH = NPC // 128
    NG = M * NPC
    DUM = npc  # first pad row of each core's shard (zeros + bias content)

    src = np.concatenate([edge_index[0], np.arange(N, dtype=np.int64)])
    dst = np.concatenate([edge_index[1], np.arange(N, dtype=np.int64)])

    # ---- attention folding ----
    v = np.asarray(att[0], np.float64)
    posm = v >= 0
    perm = np.argsort(~posm, kind="stable")
    PP = int(posm.sum())
    assert 0 < PP < F, f"degenerate attention sign split PP={PP}"
    absv = np.abs(v[perm])
    Wg_l = (Wl[:, perm] * absv[None, :]).astype(np.float32)
    bg_l = (bl[perm] * absv).astype(np.float32)
    Wg_r = (Wr[:, perm] * absv[None, :]).astype(np.float32)
    bg_r = (br[perm] * absv).astype(np.float32)

    # ---- per-core grid packing (whole segments, quad-padded, chunk-local) ----
    deg = np.bincount(dst, minlength=N)
    assert deg.min() >= 1
    qlen_all = ((deg + 3) // 4) * 4
    assert qlen_all.max() <= KC, f"segment of {qlen_all.max()} slots > KC={KC}"

    nch = 0
    binof = np.zeros(N, np.int64)
    offof = np.zeros(N, np.int64)
    for k in range(M):
        ql = qlen_all[k * npc:(k + 1) * npc]
        nbase = (int(ql.sum()) + 128 * KC - 1) // (128 * KC)
        nck, bk, ok_ = _pack_bins(ql, KC, nbase)
        nch = max(nch, nck)
        binof[k * npc:(k + 1) * npc] = bk
        offof[k * npc:(k + 1) * npc] = ok_
    L = nch * KC
    LQ = L // 4

    # per-edge slot position: sort edges by dst, enumerate within segment
    order = np.argsort(dst, kind="stable")
    ds = dst[order]
    ss = src[order]
    n_e = ds.shape[0]
    start = np.ones(n_e, bool)
    start[1:] = ds[1:] != ds[:-1]
    idxs = np.arange(n_e, dtype=np.int64)
    first = np.maximum.accumulate(np.where(start, idxs, 0))
    q = idxs - first                      # rank within segment

    seg_bin = binof[ds]                   # bin = chunk*128 + partition
    seg_c = seg_bin // 128
    seg_p = seg_bin % 128
    t = seg_c * KC + offof[ds] + q        # global slot column
    pd = seg_p
    cd = ds // npc

    srcrow = (ss // npc) * NPC + (ss % npc)

    iP = np.full((M, 128, L), 0, np.int64)
    par = np.zeros((M, 128, L), np.float32)
    mask_f = np.zeros((M, 128, L), np.float32)
    mask_r = np.zeros((M, 128, L), np.float32)
    mask_v = np.zeros((M, 128, L), np.float32)
    d_end = np.ones(n_e, bool)
    d_end[:-1] = start[1:]

    iP[cd, pd, t] = srcrow >> 1
    par[cd, pd, t] = (srcrow & 1).astype(np.float32)
    mask_f[cd, pd, t] = (~start).astype(np.float32)
    mask_r[cd, pd, t] = (~d_end).astype(np.float32)
    mask_v[cd, pd, t] = 1.0
    # pad slots keep mask_f = mask_r = mask_v = 0 and iP -> own core's DUM pair
    for k in range(M):
        pad = mask_v[k] == 0.0
        iP[k][pad] = (k * NPC + DUM) >> 1

    # quad xr index: quad (p, qd) -> local dst row of the segment covering it
    iXq = np.full((M, 128, LQ), DUM, np.int64)
    dls = ds - cd * npc
    # every quad within a segment has a real edge at its first slot
    sel = (q % 4) == 0
    iXq[cd[sel], pd[sel], (t[sel] // 4)] = dls[sel]

    def to_wrap(a):  # [128, X] -> wrap over i = col*128 + p sequence
        seq = a.T.reshape(-1)
        return _wrap_idx(seq)

    iP_w = np.stack([to_wrap(iP[k]) for k in range(M)])
    iX_w = np.stack([to_wrap(iXq[k]) for k in range(M)])

    # ---- head constants (de-permuted / de-scaled) ----
    A = bn_gamma.astype(np.float64) / np.sqrt(bn_var.astype(np.float64) + BN_EPS)
    Ap = (A[perm] / (N * absv)).astype(np.float32).reshape(F, 1)
    Bp = ((bias1 - bn_mean).astype(np.float64) * A + bn_beta)[perm] \
        .astype(np.float32).reshape(F, 1)
    Wcp = Wc[perm, :].astype(np.float32)

    # ---- per-core x^T (padded, bf16) ----
    xT = np.zeros((M, 128, NPC), BF16)
    for k in range(M):
        xT[k, :, :npc] = x[k * npc:(k + 1) * npc].T.astype(BF16)

    meta = dict(NPC=NPC, CH=CH, NG=NG, L=L, PP=PP, N=N, DUM=DUM, NCH=nch)

    in_maps = []
    for k in range(M):
        in_maps.append({
            "xT": np.ascontiguousarray(xT[k]),
            "Wgl": Wg_l.astype(BF16),
            "bgl": bg_l.reshape(1, F).astype(BF16),
            "Wgr": Wg_r.astype(BF16),
            "bgr": bg_r.reshape(1, F).astype(BF16),
            "iP": np.ascontiguousarray(iP_w[k]),
            "iX": np.ascontiguousarray(iX_w[k]),
            "par": np.ascontiguousarray(par[k]),
            "mask_f": np.ascontiguousarray(mask_f[k]),
            "mask_r": np.ascontiguousarray(mask_r[k]),
            "mask_v": np.ascontiguousarray(mask_v[k]),
            "Ap": Ap,
            "Bp": Bp,
            "Wcp": Wcp,
            "bc": bc.reshape(1, NCLS).astype(np.float32),
        })
    return in_maps, meta


def build(meta):
    NPC, CH, NG, L, PP, DUM, NCH = (
        meta[k] for k in ("NPC", "CH", "NG", "L", "PP", "DUM", "NCH"))
    LQ = L // 4
    NI = KC * 128           # pair-gather indices per chunk
    NIQ = (KC // 4) * 128   # quad-gather indices per chunk
    LW = (L * 128) // 16
    LWQ = (LQ * 128) // 16
    W2 = 2 * F * KC

    dt = mybir.dt
    alu = mybir.AluOpType
    act = mybir.ActivationFunctionType
    rg = [list(range(M))]

    nc = bacc.Bacc("TRN2", target_bir_lowering=False, debug=False, num_devices=M)

    def p_in(name, shape, d):
        return nc.dram_tensor(name, shape, d, kind="ExternalInput").ap()

    xT = p_in("xT", [128, NPC], dt.bfloat16)
    Wgl = p_in("Wgl", [F, F], dt.bfloat16)
    bgl = p_in("bgl", [1, F], dt.bfloat16)
    Wgr = p_in("Wgr", [F, F], dt.bfloat16)
    bgr = p_in("bgr", [1, F], dt.bfloat16)
    iP = p_in("iP", [128, LW], dt.int16)
    iX = p_in("iX", [128, LWQ], dt.int16)
    par = p_in("par", [128, L], dt.float32)
    mask_f = p_in("mask_f", [128, L], dt.float32)
    mask_r = p_in("mask_r", [128, L], dt.float32)
    mask_v = p_in("mask_v", [128, L], dt.float32)
    Ap = p_in("Ap", [F, 1], dt.float32)
    Bp = p_in("Bp", [F, 1], dt.float32)
    Wcp = p_in("Wcp", [F, NCLS], dt.float32)
    bc = p_in("bc", [1, NCLS], dt.float32)
    out = nc.dram_tensor("out", [1, NCLS], dt.float32, kind="ExternalOutput").ap()

    with tile.TileContext(nc) as tc:
        with (
            tc.tile_pool(name="dram", bufs=1, space="DRAM") as dpool,
            tc.tile_pool(name="sbp", bufs=1) as sbp,
            tc.tile_pool(name="sbw", bufs=2) as sbw,
            tc.tile_pool(name="ps2", bufs=2, space="PSUM") as pp,
            tc.tile_pool(name="ps1", bufs=1, space="PSUM") as pp1,
        ):
            xg_loc = dpool.tile([NPC, F], dt.bfloat16)
            xr_mini = dpool.tile([NPC, F], dt.bfloat16)
            xrq = dpool.tile([NPC, 8 * F], dt.bfloat16)
            xg_full = dpool.tile([NG, F], dt.bfloat16, addr_space="Shared")
            part_loc = dpool.tile([1, F], dt.float32)
            pooled = dpool.tile([1, F], dt.float32, addr_space="Shared")

            # ---- persistent SBUF ----
            xT_sb = sbp.tile([128, NPC], dt.bfloat16)
            nc.sync.dma_start(xT_sb[:], xT)
            wt = {}
            for nm, apin, sh in (("Wgl", Wgl, [F, F]), ("bgl", bgl, [1, F]),
                                 ("Wgr", Wgr, [F, F]), ("bgr", bgr, [1, F])):
                tl = sbp.tile(sh, dt.bfloat16, tag=nm)
                nc.sync.dma_start(tl[:], apin)
                wt[nm] = tl
            ones_sb = sbp.tile([1, F], dt.bfloat16)
            nc.vector.memset(ones_sb[:], 1.0)

            iP_sb = sbp.tile([128, LW], dt.int16)
            nc.sync.dma_start(iP_sb[:], iP)
            iX_sb = sbp.tile([128, LWQ], dt.int16)
            nc.sync.dma_start(iX_sb[:], iX)
            par_sb = sbp.tile([128, L], dt.float32)
            nc.sync.dma_start(par_sb[:], par)
            mf_sb = sbp.tile([128, L], dt.float32)
            nc.sync.dma_start(mf_sb[:], mask_f)
            mr_sb = sbp.tile([128, L], dt.float32)
            nc.sync.dma_start(mr_sb[:], mask_r)
            mv_sb = sbp.tile([128, L], dt.float32)
            nc.sync.dma_start(mv_sb[:], mask_v)

            # ---- stage A: xl table + AllGather first (gates the pair gathers);
            # xr table + quad-dup built while the AllGather runs ----
            for ci in range(CH):
                lhs = xT_sb[:, 128 * ci:128 * (ci + 1)]
                ps = pp.tile([128, F], dt.float32, tag="psA")
                nc.tensor.matmul(ps[:], lhsT=lhs, rhs=wt["Wgl"][:],
                                 start=True, stop=False)
                nc.tensor.matmul(ps[:], lhsT=ones_sb[:], rhs=wt["bgl"][:],
                                 start=False, stop=True)
                ob = sbw.tile([128, F], dt.bfloat16, tag="stA")
                nc.vector.tensor_copy(ob[:], ps[:])
                nc.sync.dma_start(xg_loc[128 * ci:128 * (ci + 1), :], ob[:])
            nc.gpsimd.collective_compute(
                "AllGather", alu.bypass, replica_groups=rg,
                ins=[xg_loc.opt()], outs=[xg_full.opt()])

            for ci in range(CH):
                lhs = xT_sb[:, 128 * ci:128 * (ci + 1)]
                ps = pp.tile([128, F], dt.float32, tag="psA")
                nc.tensor.matmul(ps[:], lhsT=lhs, rhs=wt["Wgr"][:],
                                 start=True, stop=False)
                nc.tensor.matmul(ps[:], lhsT=ones_sb[:], rhs=wt["bgr"][:],
                                 start=False, stop=True)
                ob = sbw.tile([128, F], dt.bfloat16, tag="stA")
                nc.vector.tensor_copy(ob[:], ps[:])
                nc.sync.dma_start(xr_mini[128 * ci:128 * (ci + 1), :], ob[:])
            # duplicate xr rows 8x: quad table row j = [xr_j]*8
            for i in range(8):
                nc.sync.dma_start(
                    xrq[:].rearrange("a (e f) -> a e f", e=8)[:, i, :], xr_mini[:])

            tab_pair = xg_full[:].rearrange("(a two) f -> a (two f)", two=2)

            # ---- fused pass over chunks ----
            def gather_pair(dst_tile, c):
                nc.gpsimd.dma_gather(
                    out_ap=dst_tile[:].rearrange("p (b f) -> p b f", f=2 * F),
                    in_ap=tab_pair,
                    idxs_ap=iP_sb[:, (NI // 16) * c:(NI // 16) * (c + 1)],
                    num_idxs=NI, num_idxs_reg=NI, elem_size=2 * F,
                    single_packet=False)

            def gather_quad(dst_tile, c):
                nc.gpsimd.dma_gather(
                    out_ap=dst_tile[:].rearrange("p (b f) -> p b f", f=8 * F),
                    in_ap=xrq[:],
                    idxs_ap=iX_sb[:, (NIQ // 16) * c:(NIQ // 16) * (c + 1)],
                    num_idxs=NIQ, num_idxs_reg=NIQ, elem_size=8 * F,
                    single_packet=False)

            LEAD = min(2, NCH)
            gx_tiles = {}
            for c in range(LEAD):
                gx = sbw.tile([128, W2], dt.bfloat16, tag="gx", bufs=3)
                gather_quad(gx, c)
                gx_tiles[c] = gx

            pacc = pp1.tile([F, 1], dt.float32, tag="pacc")
            for c in range(NCH):
                sl = slice(c * KC, (c + 1) * KC)
                gp = sbw.tile([128, W2], dt.bfloat16, tag="gp", bufs=3)
                gather_pair(gp, c)
                if c + LEAD < NCH:
                    gxn = sbw.tile([128, W2], dt.bfloat16, tag="gx", bufs=3)
                    gather_quad(gxn, c + LEAD)
                    gx_tiles[c + LEAD] = gxn
                gx = gx_tiles.pop(c)

                # z = gl + gr ; lrelu(z) = max(z, 0.2 z)   (all in place)
                nc.vector.tensor_tensor(out=gx[:], in0=gx[:], in1=gp[:],
                                        op=alu.add)
                nc.vector.scalar_tensor_tensor(
                    out=gx[:], in0=gx[:], scalar=0.2, in1=gx[:],
                    op0=alu.mult, op1=alu.max)
                m3 = gx[:].rearrange("p (k f) -> p k f", f=2 * F)
                lgp = sbw.tile([128, KC], dt.float32, tag="lgp")
                lgn = sbw.tile([128, KC], dt.float32, tag="lgn")
                lgp1 = sbw.tile([128, KC], dt.float32, tag="lgp1")
                lgn1 = sbw.tile([128, KC], dt.float32, tag="lgn1")
                nc.vector.tensor_reduce(lgp[:], m3[:, :, 0:PP],
                                        axis=mybir.AxisListType.X, op=alu.add)
                nc.vector.tensor_reduce(lgn[:], m3[:, :, PP:F],
                                        axis=mybir.AxisListType.X, op=alu.add)
                nc.vector.tensor_reduce(lgp1[:], m3[:, :, F:F + PP],
                                        axis=mybir.AxisListType.X, op=alu.add)
                nc.vector.tensor_reduce(lgn1[:], m3[:, :, F + PP:2 * F],
                                        axis=mybir.AxisListType.X, op=alu.add)
                # a = p0-n0 ; b = p1-n1 ; logit = a + par*(b-a)
                nc.vector.tensor_tensor(out=lgp[:], in0=lgp[:], in1=lgn[:],
                                        op=alu.subtract)
                nc.vector.tensor_tensor(out=lgp1[:], in0=lgp1[:], in1=lgn1[:],
                                        op=alu.subtract)
                nc.vector.tensor_tensor(out=lgp1[:], in0=lgp1[:], in1=lgp[:],
                                        op=alu.subtract)
                nc.vector.tensor_tensor(out=lgp1[:], in0=lgp1[:],
                                        in1=par_sb[:, sl], op=alu.mult)
                nc.vector.tensor_tensor(out=lgn[:], in0=lgp[:],
                                        in1=lgp1[:], op=alu.add)

                # chunk-local segment softmax
                E = sbw.tile([128, KC], dt.float32, tag="E")
                S = sbw.tile([128, KC], dt.float32, tag="S")
                D = sbw.tile([128, KC], dt.float32, tag="D")
                nc.scalar.activation(E[:], lgn[:], act.Exp)
                nc.vector.tensor_tensor_scan(
                    out=S[:], data0=mf_sb[:, sl], data1=E[:], initial=0.0,
                    op0=alu.mult, op1=alu.add)
                nc.vector.tensor_tensor_scan(
                    out=D[:, ::-1], data0=mr_sb[:, sl][:, ::-1],
                    data1=S[:, ::-1], initial=0.0, op0=alu.mult, op1=alu.max)
                nc.vector.reciprocal(D[:], D[:])
                nc.vector.tensor_tensor(out=E[:], in0=E[:], in1=D[:],
                                        op=alu.mult)
                nc.vector.tensor_tensor(out=E[:], in0=E[:], in1=mv_sb[:, sl],
                                        op=alu.mult)
                # w1 = w*par ; w0 = w - w1
                nc.vector.tensor_tensor(out=S[:], in0=E[:], in1=par_sb[:, sl],
                                        op=alu.mult)
                nc.vector.tensor_tensor(out=D[:], in0=E[:], in1=S[:],
                                        op=alu.subtract)
                wb0 = sbw.tile([128, KC], dt.bfloat16, tag="wb0")
                wb1 = sbw.tile([128, KC], dt.bfloat16, tag="wb1")
                nc.vector.tensor_copy(wb0[:], D[:])
                nc.vector.tensor_copy(wb1[:], S[:])

                # weighted sum while gp is in SBUF
                for b in range(KC):
                    nc.tensor.matmul(
                        pacc[:], lhsT=gp[:, 2 * F * b:2 * F * b + F],
                        rhs=wb0[:, b:b + 1], start=(c == 0 and b == 0),
                        stop=False)
                    nc.tensor.matmul(
                        pacc[:], lhsT=gp[:, 2 * F * b + F:2 * F * (b + 1)],
                        rhs=wb1[:, b:b + 1], start=False,
                        stop=(c == NCH - 1 and b == KC - 1))

            part_sb = sbp.tile([F, 1], dt.float32)
            nc.vector.tensor_copy(part_sb[:], pacc[:])
            nc.sync.dma_start(part_loc[:].rearrange("o f -> f o"), part_sb[:])

            nc.gpsimd.collective_compute(
                "AllReduce", alu.add, replica_groups=rg,
                ins=[part_loc.opt()], outs=[pooled.opt()])

            # ---- head ----
            pool_sb = sbp.tile([F, 1], dt.float32)
            nc.sync.dma_start(pool_sb[:], pooled[:].rearrange("o f -> f o"))
            Ap_sb = sbp.tile([F, 1], dt.float32)
            nc.sync.dma_start(Ap_sb[:], Ap)
            Bp_sb = sbp.tile([F, 1], dt.float32)
            nc.sync.dma_start(Bp_sb[:], Bp)
            Wc_sb = sbp.tile([F, NCLS], dt.float32)
            nc.sync.dma_start(Wc_sb[:], Wcp)
            bc_sb = sbp.tile([1, NCLS], dt.float32)
            nc.sync.dma_start(bc_sb[:], bc)
            h_sb = sbp.tile([F, 1], dt.float32)
            nc.vector.scalar_tensor_tensor(
                out=h_sb[:], in0=pool_sb[:], scalar=Ap_sb[:, 0:1], in1=Bp_sb[:],
                op0=alu.mult, op1=alu.add)
            one1 = sbp.tile([1, 1], dt.float32)
            nc.vector.memset(one1[:], 1.0)
            hp = pp1.tile([1, NCLS], dt.float32, tag="hp")
            nc.tensor.matmul(hp[:], lhsT=h_sb[:], rhs=Wc_sb[:], start=True,
                             stop=False)
            nc.tensor.matmul(hp[:], lhsT=one1[:], rhs=bc_sb[:], start=False,
                             stop=True)
            eh = sbp.tile([1, NCLS], dt.float32)
            nc.scalar.activation(eh[:], hp[:], act.Exp)
            den = sbp.tile([1, 1], dt.float32)
            nc.vector.tensor_reduce(den[:], eh[:], axis=mybir.AxisListType.X,
                                    op=alu.add)
            rden = sbp.tile([1, 1], dt.float32)
            nc.vector.reciprocal(rden[:], den[:])
            osb = sbp.tile([1, NCLS], dt.float32)
            nc.vector.tensor_scalar(out=osb[:], in0=eh[:], scalar1=rden[:, 0:1],
                                    scalar2=None, op0=alu.mult)
            nc.sync.dma_start(out, osb[:])

    nc.compile()
    return nc


# --------------------------------------------------------------------------
# public entry point
# --------------------------------------------------------------------------

_CACHE = {}


def _install_ntff_hook():
    """Provide antenv.axon_hooks + the ctypes NTFF hook when the image lacks
    them, so run_bass_kernel_spmd(trace=True) can capture exec_time_ns."""
    import contextlib
    import ctypes
    import sys
    import types

    try:
        import antenv.axon_hooks  # noqa: F401
        return
    except ImportError:
        pass
    try:
        import antenv
    except ImportError:
        return
    holder = [None]
    mod = types.ModuleType("antenv.axon_hooks")
    mod.set_axon_ntff_profile_hook = lambda h: holder.__setitem__(0, h)
    mod.get_axon_ntff_profile_hook = lambda: holder[0]
    sys.modules["antenv.axon_hooks"] = mod
    antenv.axon_hooks = mod

    so_path = "/opt/axon/libaxon_pjrt.so"
    if os.path.exists(so_path):
        lib = ctypes.CDLL(so_path)
        if hasattr(lib, "axon_start_nrt_profile"):
            lib.axon_start_nrt_profile.argtypes = [
                ctypes.POINTER(ctypes.c_int64), ctypes.c_size_t]
            lib.axon_start_nrt_profile.restype = ctypes.c_int64
            lib.axon_stop_nrt_profile.argtypes = [ctypes.c_char_p]
            lib.axon_stop_nrt_profile.restype = ctypes.c_int64

            @contextlib.contextmanager
            def _hook(output_dir, device_ids):
                import jax
                jax.devices()
                if device_ids:
                    ids = (ctypes.c_int64 * len(device_ids))(*device_ids)
                    rc = lib.axon_start_nrt_profile(ids, len(device_ids))
                else:
                    rc = lib.axon_start_nrt_profile(None, 0)
                if rc != 0:
                    raise RuntimeError(f"axon_start_nrt_profile rc={rc}")
                try:
                    yield
                finally:
                    n = lib.axon_stop_nrt_profile(str(output_dir).encode())
                    print(f"ntff profile: {n} file(s) -> {output_dir}")

            mod.set_axon_ntff_profile_hook(_hook)

    import concourse.bass_utils as bu
    bu.upload_artifacts = lambda tmpdir: "local://" + str(tmpdir)


def kernel(**inputs):
    from concourse.bass_utils import run_bass_kernel_spmd

    if bool(int(os.environ.get("KERNEL_TRACE", "0"))):
        _install_ntff_hook()
    inputs = {k: np.asarray(v) for k, v in inputs.items()}
    in_maps, meta = prep_host(**inputs)
    key = tuple(sorted(meta.items()))
    if key not in _CACHE:
        _CACHE[key] = build(meta)
    nc = _CACHE[key]
    res = run_bass_kernel_spmd(nc, in_maps, core_ids=list(range(M)),
                               trace=bool(int(os.environ.get("KERNEL_TRACE", "0"))))
    if getattr(res, "exec_time_ns", None) is not None:
        print(f"HW exec time: {res.exec_time_ns} ns")
    return np.asarray(res.results[0]["out"]).astype(np.float32)


# revision 13
# speedup vs baseline: 1.7123x; 1.1034x over previous
"""Distributed Trainium2 kernel for a GATv2 layer + BN + global-mean-pool + classifier.

Math (reference, heads=1):
    xl = x@Wl + bl ; xr = x@Wr + br
    logit_e = att . leaky_relu(xl[src_e] + xr[dst_e], 0.2)
    a_e     = segment_softmax(logit_e over dst)
    out_i   = sum_{e: dst=i} a_e * xl[src_e] ; out = out + bias1
    h       = BN(out) ; g = mean_i h ; y = softmax(g@Wc + bc)

The output is a global mean over nodes and BN is affine per feature, so
per-node outputs never materialize:
    y = softmax( ((S/N)*A + B) @ Wc + bc ),  S = sum_e a_e * xl[src_e],
    A = gamma/sqrt(var+eps), B = (bias1 - mu)*A + beta.

Attention weights v = att are folded into the gather tables host-side:
    v_f * lrelu(z_f) = sign_f * lrelu(|v_f| z_f)
with features permuted so positive-sign features occupy columns [0,PP).

Distribution over 8 cores: nodes sharded contiguously; edges sharded by dst.
Per core the edges are packed, whole dst-segments at a time, into
(chunk, partition) bins of KC slots — no segment straddles a chunk — and
each segment is padded to a multiple of 4 slots.  Per chunk:
  * xl[src] pair-rows come from one bulk dma_gather (int16 idx, 512B elems)
  * xr[dst] comes from a QUAD dma_gather (2048B elems = 4 slots' worth of
    duplicated xr rows -> 4x fewer descriptors; SWDGE cost is per descriptor)
  * logits: z = gl+gr ; lrelu(z) = max(z, 0.2z) in place; 4 masked reduces
  * segment softmax via chunk-local masked fwd add-scan + rev max-scan
  * weighted sum: per-slot [128,F]^T @ w matmuls accumulate into PSUM while
    the gathered rows are still in SBUF (single pass, no DRAM stash).
AllReduce of the [F] partial + a tiny head finishes.
"""

import math
import os

import ml_dtypes
import numpy as np

import concourse.bass as bass
import concourse.bacc as bacc
import concourse.mybir as mybir
import concourse.tile as tile

M = 8  # cores
F = 128
NCLS = 5
BN_EPS = 1e-5
KC = 40  # slot-columns per chunk

BF16 = ml_dtypes.bfloat16


def _wrap_idx(seq):
    """[N] int array -> [128, N//16] int16 wrap layout (16-partition groups,
    replicated across the 8 gpsimd cores)."""
    n = seq.shape[0]
    assert n % 16 == 0
    w = seq.reshape(n // 16, 16).T.astype(np.int16)
    return np.tile(w, (8, 1))


def _pack_bins(qlen, kc, nbase):
    """Pack segments (sizes qlen, each <= kc) whole into 128*nch bins of
    capacity kc.  First-fit-decreasing, preferring earlier chunks (and within
    a chunk, fuller bins) so the final chunk stays light — it sets the
    pipeline tail.  Returns (nch, bin_of_seg, off_of_seg); bin = chunk*128+p."""
    order = np.argsort(-qlen, kind="stable")
    nch = nbase
    while True:
        nbins = 128 * nch
        loads = np.zeros(nbins, np.int64)
        chunk_of = np.arange(nbins) // 128
        binof = np.zeros(qlen.shape[0], np.int64)
        offof = np.zeros(qlen.shape[0], np.int64)
        ok = True
        for g in order:
            q = qlen[g]
            cand = np.nonzero(loads + q <= kc)[0]
            if cand.size == 0:
                ok = False
                break
            # earliest chunk; within it the fullest bin
            key = chunk_of[cand] * (kc + 1) - loads[cand]
            b = int(cand[np.argmin(key)])
            binof[g] = b
            offof[g] = loads[b]
            loads[b] += q
        if ok:
            return nch, binof, offof
        nch += 1


def prep_host(x, edge_index, Wl, bl, Wr, br, att, bias1,
              bn_gamma, bn_beta, bn_mean, bn_var, Wc, bc):
    N = x.shape[0]
    npc = N // M
    assert npc * M == N
    NPC = ((npc + 1 + 127) // 128) * 128  # always >= 1 pad row (dummy)
    CH = NPC // 128
    NG = M * NPC
    DUM = npc  # first pad row of each core's shard (zeros + bias content)

    src = np.concatenate([edge_index[0], np.arange(N, dtype=np.int64)])
    dst = np.concatenate([edge_index[1], np.arange(N, dtype=np.int64)])

    # ---- attention folding ----
    v = np.asarray(att[0], np.float64)
    posm = v >= 0
    perm = np.argsort(~posm, kind="stable")
    PP = int(posm.sum())
    assert 0 < PP < F, f"degenerate attention sign split PP={PP}"
    absv = np.abs(v[perm])
    Wg_l = (Wl[:, perm] * absv[None, :]).astype(np.float32)
    bg_l = (bl[perm] * absv).astype(np.float32)
    Wg_r = (Wr[:, perm] * absv[None, :]).astype(np.float32)
    bg_r = (br[perm] * absv).astype(np.float32)

    # ---- per-core grid packing (whole segments, quad-padded, chunk-local) ----
    deg = np.bincount(dst, minlength=N)
    assert deg.min() >= 1
    qlen_all = ((deg + 3) // 4) * 4
    assert qlen_all.max() <= KC, f"segment of {qlen_all.max()} slots > KC={KC}"

    nch = 0
    binof = np.zeros(N, np.int64)
    offof = np.zeros(N, np.int64)
    for k in range(M):
        ql = qlen_all[k * npc:(k + 1) * npc]
        nbase = (int(ql.sum()) + 128 * KC - 1) // (128 * KC)
        nck, bk, ok_ = _pack_bins(ql, KC, nbase)
        nch = max(nch, nck)
        binof[k * npc:(k + 1) * npc] = bk
        offof[k * npc:(k + 1) * npc] = ok_
    L = nch * KC
    LQ = L // 4

    # per-edge slot position: sort edges by dst, enumerate within segment
    order = np.argsort(dst, kind="stable")
    ds = dst[order]
    ss = src[order]
    n_e = ds.shape[0]
    start = np.ones(n_e, bool)
    start[1:] = ds[1:] != ds[:-1]
    idxs = np.arange(n_e, dtype=np.int64)
    first = np.maximum.accumulate(np.where(start, idxs, 0))
    q = idxs - first                      # rank within segment

    seg_bin = binof[ds]                   # bin = chunk*128 + partition
    seg_c = seg_bin // 128
    seg_p = seg_bin % 128
    t = seg_c * KC + offof[ds] + q        # global slot column
    pd = seg_p
    cd = ds // npc

    srcrow = (ss // npc) * NPC + (ss % npc)

    iP = np.full((M, 128, L), 0, np.int64)
    par = np.zeros((M, 128, L), np.float32)
    mask_f = np.zeros((M, 128, L), np.float32)
    mask_r = np.zeros((M, 128, L), np.float32)
    mask_v = np.zeros((M, 128, L), np.float32)
    d_end = np.ones(n_e, bool)
    d_end[:-1] = start[1:]

    iP[cd, pd, t] = srcrow >> 1
    par[cd, pd, t] = (srcrow & 1).astype(np.float32)
    mask_f[cd, pd, t] = (~start).astype(np.float32)
    mask_r[cd, pd, t] = (~d_end).astype(np.float32)
    mask_v[cd, pd, t] = 1.0
    # pad slots keep mask_f = mask_r = mask_v = 0 and iP -> own core's DUM pair
    for k in range(M):
        pad = mask_v[k] == 0.0
        iP[k][pad] = (k * NPC + DUM) >> 1

    # quad xr index: quad (p, qd) -> local dst row of the segment covering it
    iXq = np.full((M, 128, LQ), DUM, np.int64)
    dls = ds - cd * npc
    # every quad within a segment has a real edge at its first slot
    sel = (q % 4) == 0
    iXq[cd[sel], pd[sel], (t[sel] // 4)] = dls[sel]

    def to_wrap(a):  # [128, X] -> wrap over i = col*128 + p sequence
        seq = a.T.reshape(-1)
        return _wrap_idx(seq)

    iP_w = np.stack([to_wrap(iP[k]) for k in range(M)])
    iX_w = np.stack([to_wrap(iXq[k]) for k in range(M)])

    # ---- head constants (de-permuted / de-scaled) ----
    A = bn_gamma.astype(np.float64) / np.sqrt(bn_var.astype(np.float64) + BN_EPS)
    Ap = (A[perm] / (N * absv)).astype(np.float32).reshape(F, 1)
    Bp = ((bias1 - bn_mean).astype(np.float64) * A + bn_beta)[perm] \
        .astype(np.float32).reshape(F, 1)
    Wcp = Wc[perm, :].astype(np.float32)

    # ---- per-core x^T (padded, bf16) ----
    xT = np.zeros((M, 128, NPC), BF16)
    for k in range(M):
        xT[k, :, :npc] = x[k * npc:(k + 1) * npc].T.astype(BF16)

    meta = dict(NPC=NPC, CH=CH, NG=NG, L=L, PP=PP, N=N, DUM=DUM, NCH=nch)

    in_maps = []
    for k in range(M):
        in_maps.append({
            "xT": np.ascontiguousarray(xT[k]),
            "Wgl": Wg_l.astype(BF16),
            "bgl": bg_l.reshape(1, F).astype(BF16),
            "Wgr": Wg_r.astype(BF16),
            "bgr": bg_r.reshape(1, F).astype(BF16),
            "iP": np.ascontiguousarray(iP_w[k]),
            "iX": np.ascontiguousarray(iX_w[k]),
            "par": np.ascontiguousarray(par[k]),
            "mask_f": np.ascontiguousarray(mask_f[k]),
            "mask_r": np.ascontiguousarray(mask_r[k]),
            "mask_v": np.ascontiguousarray(mask_v[k]),
            "Ap": Ap,
            "Bp": Bp,
            "Wcp": Wcp,
            "bc": bc.reshape(1, NCLS).astype(np.float32),
        })
    return in_maps, meta


def build(meta):
    NPC, CH, NG, L, PP, DUM, NCH = (
        meta[k] for k in ("NPC", "CH", "NG", "L", "PP", "DUM", "NCH"))
    LQ = L // 4
    NI = KC * 128           # pair-gather indices per chunk
    NIQ = (KC // 4) * 128   # quad-gather indices per chunk
    LW = (L * 128) // 16
    LWQ = (LQ * 128) // 16
    W2 = 2 * F * KC

    dt = mybir.dt
    alu = mybir.AluOpType
    act = mybir.ActivationFunctionType
    rg = [list(range(M))]

    nc = bacc.Bacc("TRN2", target_bir_lowering=False, debug=False, num_devices=M)

    def p_in(name, shape, d):
        return nc.dram_tensor(name, shape, d, kind="ExternalInput").ap()

    xT = p_in("xT", [128, NPC], dt.bfloat16)
    Wgl = p_in("Wgl", [F, F], dt.bfloat16)
    bgl = p_in("bgl", [1, F], dt.bfloat16)
    Wgr = p_in("Wgr", [F, F], dt.bfloat16)
    bgr = p_in("bgr", [1, F], dt.bfloat16)
    iP = p_in("iP", [128, LW], dt.int16)
    iX = p_in("iX", [128, LWQ], dt.int16)
    par = p_in("par", [128, L], dt.float32)
    mask_f = p_in("mask_f", [128, L], dt.float32)
    mask_r = p_in("mask_r", [128, L], dt.float32)
    mask_v = p_in("mask_v", [128, L], dt.float32)
    Ap = p_in("Ap", [F, 1], dt.float32)
    Bp = p_in("Bp", [F, 1], dt.float32)
    Wcp = p_in("Wcp", [F, NCLS], dt.float32)
    bc = p_in("bc", [1, NCLS], dt.float32)
    out = nc.dram_tensor("out", [1, NCLS], dt.float32, kind="ExternalOutput").ap()

    with tile.TileContext(nc) as tc:
        with (
            tc.tile_pool(name="dram", bufs=1, space="DRAM") as dpool,
            tc.tile_pool(name="sbp", bufs=1) as sbp,
            tc.tile_pool(name="sbw", bufs=2) as sbw,
            tc.tile_pool(name="ps2", bufs=2, space="PSUM") as pp,
            tc.tile_pool(name="ps1", bufs=1, space="PSUM") as pp1,
        ):
            xg_loc = dpool.tile([NPC, F], dt.bfloat16)
            xr_mini = dpool.tile([NPC, F], dt.bfloat16)
            xrq = dpool.tile([NPC, 8 * F], dt.bfloat16)
            xg_full = dpool.tile([NG, F], dt.bfloat16, addr_space="Shared")
            part_loc = dpool.tile([1, F], dt.float32)
            pooled = dpool.tile([1, F], dt.float32, addr_space="Shared")

            # ---- persistent SBUF ----
            xT_sb = sbp.tile([128, NPC], dt.bfloat16)
            nc.sync.dma_start(xT_sb[:], xT)
            wt = {}
            for nm, apin, sh in (("Wgl", Wgl, [F, F]), ("bgl", bgl, [1, F]),
                                 ("Wgr", Wgr, [F, F]), ("bgr", bgr, [1, F])):
                tl = sbp.tile(sh, dt.bfloat16, tag=nm)
                nc.sync.dma_start(tl[:], apin)
                wt[nm] = tl
            ones_sb = sbp.tile([1, F], dt.bfloat16)
            nc.vector.memset(ones_sb[:], 1.0)

            # ---- stage A: xl table + AllGather first (gates the pair gathers);
            # xr table + quad-dup built while the AllGather runs.  Table rows
            # are staged 4 matmul-chunks at a time so each DRAM write is one
            # big DMA instead of 49 small ones (sync-queue dispatch is ~1us). ----
            def build_table(wn, bn_, dst_dram):
                for g0 in range(0, CH, 4):
                    gn = min(4, CH - g0)
                    stg = sbw.tile([128, 4 * F], dt.bfloat16, tag="stg")
                    for j in range(gn):
                        ci = g0 + j
                        lhs = xT_sb[:, 128 * ci:128 * (ci + 1)]
                        ps = pp.tile([128, F], dt.float32, tag="psA")
                        nc.tensor.matmul(ps[:], lhsT=lhs, rhs=wt[wn][:],
                                         start=True, stop=False)
                        nc.tensor.matmul(ps[:], lhsT=ones_sb[:], rhs=wt[bn_][:],
                                         start=False, stop=True)
                        nc.vector.tensor_copy(stg[:, F * j:F * (j + 1)], ps[:])
                    nc.sync.dma_start(
                        dst_dram[128 * g0:128 * (g0 + gn), :]
                        .rearrange("(c p) f -> p c f", c=gn),
                        stg[:, 0:gn * F].rearrange("p (c f) -> p c f", c=gn))

            build_table("Wgl", "bgl", xg_loc)
            nc.gpsimd.collective_compute(
                "AllGather", alu.bypass, replica_groups=rg,
                ins=[xg_loc.opt()], outs=[xg_full.opt()])

            build_table("Wgr", "bgr", xr_mini)
            # duplicate xr rows 8x: quad table row j = [xr_j]*8
            for i in range(8):
                nc.sync.dma_start(
                    xrq[:].rearrange("a (e f) -> a e f", e=8)[:, i, :], xr_mini[:])

            # grid metadata loads: only needed once chunk-0 compute starts
            iP_sb = sbp.tile([128, LW], dt.int16)
            nc.sync.dma_start(iP_sb[:], iP)
            iX_sb = sbp.tile([128, LWQ], dt.int16)
            nc.sync.dma_start(iX_sb[:], iX)
            par_sb = sbp.tile([128, L], dt.float32)
            nc.sync.dma_start(par_sb[:], par)
            mf_sb = sbp.tile([128, L], dt.float32)
            nc.sync.dma_start(mf_sb[:], mask_f)
            mr_sb = sbp.tile([128, L], dt.float32)
            nc.sync.dma_start(mr_sb[:], mask_r)
            mv_sb = sbp.tile([128, L], dt.float32)
            nc.sync.dma_start(mv_sb[:], mask_v)

            tab_pair = xg_full[:].rearrange("(a two) f -> a (two f)", two=2)

            # ---- fused pass over chunks ----
            def gather_pair(dst_tile, c):
                nc.gpsimd.dma_gather(
                    out_ap=dst_tile[:].rearrange("p (b f) -> p b f", f=2 * F),
                    in_ap=tab_pair,
                    idxs_ap=iP_sb[:, (NI // 16) * c:(NI // 16) * (c + 1)],
                    num_idxs=NI, num_idxs_reg=NI, elem_size=2 * F,
                    single_packet=False)

            def gather_quad(dst_tile, c):
                nc.gpsimd.dma_gather(
                    out_ap=dst_tile[:].rearrange("p (b f) -> p b f", f=8 * F),
                    in_ap=xrq[:],
                    idxs_ap=iX_sb[:, (NIQ // 16) * c:(NIQ // 16) * (c + 1)],
                    num_idxs=NIQ, num_idxs_reg=NIQ, elem_size=8 * F,
                    single_packet=False)

            LEAD = min(2, NCH)
            gx_tiles = {}
            for c in range(LEAD):
                gx = sbw.tile([128, W2], dt.bfloat16, tag="gx", bufs=3)
                gather_quad(gx, c)
                gx_tiles[c] = gx

            pacc = pp1.tile([F, 1], dt.float32, tag="pacc")
            for c in range(NCH):
                sl = slice(c * KC, (c + 1) * KC)
                gp = sbw.tile([128, W2], dt.bfloat16, tag="gp", bufs=4)
                gather_pair(gp, c)
                if c + LEAD < NCH:
                    gxn = sbw.tile([128, W2], dt.bfloat16, tag="gx", bufs=3)
                    gather_quad(gxn, c + LEAD)
                    gx_tiles[c + LEAD] = gxn
                gx = gx_tiles.pop(c)

                # z = gl + gr (vector) ; lrelu on the scalar engine, in place
                nc.vector.tensor_tensor(out=gx[:], in0=gx[:], in1=gp[:],
                                        op=alu.add)
                nc.scalar.activation(gx[:], gx[:], act.Lrelu, alpha=0.2)
                m3 = gx[:].rearrange("p (k f) -> p k f", f=2 * F)
                lgp = sbw.tile([128, KC], dt.float32, tag="lgp")
                lgn = sbw.tile([128, KC], dt.float32, tag="lgn")
                lgp1 = sbw.tile([128, KC], dt.float32, tag="lgp1")
                lgn1 = sbw.tile([128, KC], dt.float32, tag="lgn1")
                nc.vector.tensor_reduce(lgp[:], m3[:, :, 0:PP],
                                        axis=mybir.AxisListType.X, op=alu.add)
                nc.vector.tensor_reduce(lgn[:], m3[:, :, PP:F],
                                        axis=mybir.AxisListType.X, op=alu.add)
                nc.vector.tensor_reduce(lgp1[:], m3[:, :, F:F + PP],
                                        axis=mybir.AxisListType.X, op=alu.add)
                nc.vector.tensor_reduce(lgn1[:], m3[:, :, F + PP:2 * F],
                                        axis=mybir.AxisListType.X, op=alu.add)
                # a = p0-n0 ; b = p1-n1 ; logit = a + par*(b-a)
                nc.vector.tensor_tensor(out=lgp[:], in0=lgp[:], in1=lgn[:],
                                        op=alu.subtract)
                nc.vector.tensor_tensor(out=lgp1[:], in0=lgp1[:], in1=lgn1[:],
                                        op=alu.subtract)
                nc.vector.tensor_tensor(out=lgp1[:], in0=lgp1[:], in1=lgp[:],
                                        op=alu.subtract)
                nc.vector.tensor_tensor(out=lgp1[:], in0=lgp1[:],
                                        in1=par_sb[:, sl], op=alu.mult)
                nc.vector.tensor_tensor(out=lgn[:], in0=lgp[:],
                                        in1=lgp1[:], op=alu.add)

                # chunk-local segment softmax
                E = sbw.tile([128, KC], dt.float32, tag="E")
                S = sbw.tile([128, KC], dt.float32, tag="S")
                D = sbw.tile([128, KC], dt.float32, tag="D")
                nc.scalar.activation(E[:], lgn[:], act.Exp)
                nc.vector.tensor_tensor_scan(
                    out=S[:], data0=mf_sb[:, sl], data1=E[:], initial=0.0,
                    op0=alu.mult, op1=alu.add)
                nc.vector.tensor_tensor_scan(
                    out=D[:, ::-1], data0=mr_sb[:, sl][:, ::-1],
                    data1=S[:, ::-1], initial=0.0, op0=alu.mult, op1=alu.max)
                nc.vector.reciprocal(D[:], D[:])
                nc.vector.tensor_tensor(out=E[:], in0=E[:], in1=D[:],
                                        op=alu.mult)
                nc.vector.tensor_tensor(out=E[:], in0=E[:], in1=mv_sb[:, sl],
                                        op=alu.mult)
                # w1 = w*par ; w0 = w - w1
                nc.vector.tensor_tensor(out=S[:], in0=E[:], in1=par_sb[:, sl],
                                        op=alu.mult)
                nc.vector.tensor_tensor(out=D[:], in0=E[:], in1=S[:],
                                        op=alu.subtract)
                wb0 = sbw.tile([128, KC], dt.bfloat16, tag="wb0")
                wb1 = sbw.tile([128, KC], dt.bfloat16, tag="wb1")
                nc.vector.tensor_copy(wb0[:], D[:])
                nc.vector.tensor_copy(wb1[:], S[:])

                # weighted sum while gp is in SBUF
                for b in range(KC):
                    nc.tensor.matmul(
                        pacc[:], lhsT=gp[:, 2 * F * b:2 * F * b + F],
                        rhs=wb0[:, b:b + 1], start=(c == 0 and b == 0),
                        stop=False)
                    nc.tensor.matmul(
                        pacc[:], lhsT=gp[:, 2 * F * b + F:2 * F * (b + 1)],
                        rhs=wb1[:, b:b + 1], start=False,
                        stop=(c == NCH - 1 and b == KC - 1))

            part_sb = sbp.tile([F, 1], dt.float32)
            nc.vector.tensor_copy(part_sb[:], pacc[:])
            nc.sync.dma_start(part_loc[:].rearrange("o f -> f o"), part_sb[:])

            nc.gpsimd.collective_compute(
                "AllReduce", alu.add, replica_groups=rg,
                ins=[part_loc.opt()], outs=[pooled.opt()])

            # ---- head ----
            pool_sb = sbp.tile([F, 1], dt.float32)
            nc.sync.dma_start(pool_sb[:], pooled[:].rearrange("o f -> f o"))
            Ap_sb = sbp.tile([F, 1], dt.float32)
            nc.sync.dma_start(Ap_sb[:], Ap)
            Bp_sb = sbp.tile([F, 1], dt.float32)
            nc.sync.dma_start(Bp_sb[:], Bp)
            Wc_sb = sbp.tile([F, NCLS], dt.float32)
            nc.sync.dma_start(Wc_sb[:], Wcp)
            bc_sb = sbp.tile([1, NCLS], dt.float32)
            nc.sync.dma_start(bc_sb[:], bc)
            h_sb = sbp.tile([F, 1], dt.float32)
            nc.vector.scalar_tensor_tensor(
                out=h_sb[:], in0=pool_sb[:], scalar=Ap_sb[:, 0:1], in1=Bp_sb[:],
                op0=alu.mult, op1=alu.add)
            one1 = sbp.tile([1, 1], dt.float32)
            nc.vector.memset(one1[:], 1.0)
            hp = pp1.tile([1, NCLS], dt.float32, tag="hp")
            nc.tensor.matmul(hp[:], lhsT=h_sb[:], rhs=Wc_sb[:], start=True,
                             stop=False)
            nc.tensor.matmul(hp[:], lhsT=one1[:], rhs=bc_sb[:], start=False,
                             stop=True)
            eh = sbp.tile([1, NCLS], dt.float32)
            nc.scalar.activation(eh[:], hp[:], act.Exp)
            den = sbp.tile([1, 1], dt.float32)
            nc.vector.tensor_reduce(den[:], eh[:], axis=mybir.AxisListType.X,
                                    op=alu.add)
            rden = sbp.tile([1, 1], dt.float32)
            nc.vector.reciprocal(rden[:], den[:])
            osb = sbp.tile([1, NCLS], dt.float32)
            nc.vector.tensor_scalar(out=osb[:], in0=eh[:], scalar1=rden[:, 0:1],
                                    scalar2=None, op0=alu.mult)
            nc.sync.dma_start(out, osb[:])

    nc.compile()
    return nc


# --------------------------------------------------------------------------
# public entry point
# --------------------------------------------------------------------------

_CACHE = {}


def _install_ntff_hook():
    """Provide antenv.axon_hooks + the ctypes NTFF hook when the image lacks
    them, so run_bass_kernel_spmd(trace=True) can capture exec_time_ns."""
    import contextlib
    import ctypes
    import sys
    import types

    try:
        import antenv.axon_hooks  # noqa: F401
        return
    except ImportError:
        pass
    try:
        import antenv
    except ImportError:
        return
    holder = [None]
    mod = types.ModuleType("antenv.axon_hooks")
    mod.set_axon_ntff_profile_hook = lambda h: holder.__setitem__(0, h)
    mod.get_axon_ntff_profile_hook = lambda: holder[0]
    sys.modules["antenv.axon_hooks"] = mod
    antenv.axon_hooks = mod

    so_path = "/opt/axon/libaxon_pjrt.so"
    if os.path.exists(so_path):
        lib = ctypes.CDLL(so_path)
        if hasattr(lib, "axon_start_nrt_profile"):
            lib.axon_start_nrt_profile.argtypes = [
                ctypes.POINTER(ctypes.c_int64), ctypes.c_size_t]
            lib.axon_start_nrt_profile.restype = ctypes.c_int64
            lib.axon_stop_nrt_profile.argtypes = [ctypes.c_char_p]
            lib.axon_stop_nrt_profile.restype = ctypes.c_int64

            @contextlib.contextmanager
            def _hook(output_dir, device_ids):
                import jax
                jax.devices()
                if device_ids:
                    ids = (ctypes.c_int64 * len(device_ids))(*device_ids)
                    rc = lib.axon_start_nrt_profile(ids, len(device_ids))
                else:
                    rc = lib.axon_start_nrt_profile(None, 0)
                if rc != 0:
                    raise RuntimeError(f"axon_start_nrt_profile rc={rc}")
                try:
                    yield
                finally:
                    n = lib.axon_stop_nrt_profile(str(output_dir).encode())
                    print(f"ntff profile: {n} file(s) -> {output_dir}")

            mod.set_axon_ntff_profile_hook(_hook)

    import concourse.bass_utils as bu
    bu.upload_artifacts = lambda tmpdir: "local://" + str(tmpdir)


def kernel(**inputs):
    from concourse.bass_utils import run_bass_kernel_spmd

    if bool(int(os.environ.get("KERNEL_TRACE", "0"))):
        _install_ntff_hook()
    inputs = {k: np.asarray(v) for k, v in inputs.items()}
    in_maps, meta = prep_host(**inputs)
    key = tuple(sorted(meta.items()))
    if key not in _CACHE:
        _CACHE[key] = build(meta)
    nc = _CACHE[key]
    res = run_bass_kernel_spmd(nc, in_maps, core_ids=list(range(M)),
                               trace=bool(int(os.environ.get("KERNEL_TRACE", "0"))))
    if getattr(res, "exec_time_ns", None) is not None:
        print(f"HW exec time: {res.exec_time_ns} ns")
    return np.asarray(res.results[0]["out"]).astype(np.float32)


# revision 17
# speedup vs baseline: 1.7507x; 1.0224x over previous
"""Distributed Trainium2 kernel for a GATv2 layer + BN + global-mean-pool + classifier.

Math (reference, heads=1):
    xl = x@Wl + bl ; xr = x@Wr + br
    logit_e = att . leaky_relu(xl[src_e] + xr[dst_e], 0.2)
    a_e     = segment_softmax(logit_e over dst)
    out_i   = sum_{e: dst=i} a_e * xl[src_e] ; out = out + bias1
    h       = BN(out) ; g = mean_i h ; y = softmax(g@Wc + bc)

The output is a global mean over nodes and BN is affine per feature, so
per-node outputs never materialize:
    y = softmax( ((S/N)*A + B) @ Wc + bc ),  S = sum_e a_e * xl[src_e],
    A = gamma/sqrt(var+eps), B = (bias1 - mu)*A + beta.

Attention weights v = att are folded into the gather tables host-side:
    v_f * lrelu(z_f) = sign_f * lrelu(|v_f| z_f)
with features permuted so positive-sign features occupy columns [0,PP).

Distribution over 8 cores: nodes sharded contiguously; edges sharded by dst.
Per core the edges are packed, whole dst-segments at a time, into
(chunk, partition) bins of KC slots — no segment straddles a chunk — and
each segment is padded to a multiple of 4 slots.  Per chunk:
  * xl[src] pair-rows come from one bulk dma_gather (int16 idx, 512B elems)
  * xr[dst] comes from a QUAD dma_gather (2048B elems = 4 slots' worth of
    duplicated xr rows -> 4x fewer descriptors; SWDGE cost is per descriptor)
  * logits: z = gl+gr ; lrelu(z) = max(z, 0.2z) in place; 4 masked reduces
  * segment softmax via chunk-local masked fwd add-scan + rev max-scan
  * weighted sum: per-slot [128,F]^T @ w matmuls accumulate into PSUM while
    the gathered rows are still in SBUF (single pass, no DRAM stash).
AllReduce of the [F] partial + a tiny head finishes.
"""

import math
import os

import ml_dtypes
import numpy as np

import concourse.bass as bass
import concourse.bacc as bacc
import concourse.mybir as mybir
import concourse.tile as tile

M = 8  # cores
F = 128
NCLS = 5
BN_EPS = 1e-5
KC = 44  # slot-columns per chunk

BF16 = ml_dtypes.bfloat16


def _wrap_idx(seq):
    """[N] int array -> [128, N//16] int16 wrap layout (16-partition groups,
    replicated across the 8 gpsimd cores)."""
    n = seq.shape[0]
    assert n % 16 == 0
    w = seq.reshape(n // 16, 16).T.astype(np.int16)
    return np.tile(w, (8, 1))


def _pack_bins(qlen, kc, nbase):
    """Pack segments (sizes qlen, each <= kc) whole into 128*nch bins of
    capacity kc.  First-fit-decreasing, preferring earlier chunks (and within
    a chunk, fuller bins) so the final chunk stays light — it sets the
    pipeline tail.  Returns (nch, bin_of_seg, off_of_seg); bin = chunk*128+p."""
    order = np.argsort(-qlen, kind="stable")
    nch = nbase
    while True:
        nbins = 128 * nch
        loads = np.zeros(nbins, np.int64)
        chunk_of = np.arange(nbins) // 128
        binof = np.zeros(qlen.shape[0], np.int64)
        offof = np.zeros(qlen.shape[0], np.int64)
        ok = True
        for g in order:
            q = qlen[g]
            cand = np.nonzero(loads + q <= kc)[0]
            if cand.size == 0:
                ok = False
                break
            # best-fit-decreasing: fullest bin that fits (ties: earliest chunk)
            b = int(cand[np.argmax(loads[cand] * nbins - cand)])
            binof[g] = b
            offof[g] = loads[b]
            loads[b] += q
        if ok:
            return nch, binof, offof
        nch += 1


def prep_host(x, edge_index, Wl, bl, Wr, br, att, bias1,
              bn_gamma, bn_beta, bn_mean, bn_var, Wc, bc):
    N = x.shape[0]
    npc = N // M
    assert npc * M == N
    NPC = ((npc + 1 + 127) // 128) * 128  # always >= 1 pad row (dummy)
    CH = NPC // 128
    NG = M * NPC
    DUM = npc  # first pad row of each core's shard (zeros + bias content)

    src = np.concatenate([edge_index[0], np.arange(N, dtype=np.int64)])
    dst = np.concatenate([edge_index[1], np.arange(N, dtype=np.int64)])

    # ---- attention folding ----
    v = np.asarray(att[0], np.float64)
    posm = v >= 0
    perm = np.argsort(~posm, kind="stable")
    PP = int(posm.sum())
    assert 0 < PP < F, f"degenerate attention sign split PP={PP}"
    absv = np.abs(v[perm])
    Wg_l = (Wl[:, perm] * absv[None, :]).astype(np.float32)
    bg_l = (bl[perm] * absv).astype(np.float32)
    Wg_r = (Wr[:, perm] * absv[None, :]).astype(np.float32)
    bg_r = (br[perm] * absv).astype(np.float32)

    # ---- per-core grid packing (whole segments, quad-padded, chunk-local) ----
    deg = np.bincount(dst, minlength=N)
    assert deg.min() >= 1
    qlen_all = ((deg + 3) // 4) * 4
    assert qlen_all.max() <= KC, f"segment of {qlen_all.max()} slots > KC={KC}"

    nch = 0
    binof = np.zeros(N, np.int64)
    offof = np.zeros(N, np.int64)
    for k in range(M):
        ql = qlen_all[k * npc:(k + 1) * npc]
        nbase = (int(ql.sum()) + 128 * KC - 1) // (128 * KC)
        nck, bk, ok_ = _pack_bins(ql, KC, nbase)
        nch = max(nch, nck)
        binof[k * npc:(k + 1) * npc] = bk
        offof[k * npc:(k + 1) * npc] = ok_
    L = nch * KC
    LQ = L // 4

    # per-edge slot position: sort edges by dst, enumerate within segment
    order = np.argsort(dst, kind="stable")
    ds = dst[order]
    ss = src[order]
    n_e = ds.shape[0]
    start = np.ones(n_e, bool)
    start[1:] = ds[1:] != ds[:-1]
    idxs = np.arange(n_e, dtype=np.int64)
    first = np.maximum.accumulate(np.where(start, idxs, 0))
    q = idxs - first                      # rank within segment

    seg_bin = binof[ds]                   # bin = chunk*128 + partition
    seg_c = seg_bin // 128
    seg_p = seg_bin % 128
    t = seg_c * KC + offof[ds] + q        # global slot column
    pd = seg_p
    cd = ds // npc

    srcrow = (ss // npc) * NPC + (ss % npc)

    iP = np.full((M, 128, L), 0, np.int64)
    par = np.zeros((M, 128, L), np.float32)
    mask_f = np.zeros((M, 128, L), np.float32)
    mask_r = np.zeros((M, 128, L), np.float32)
    mask_v = np.zeros((M, 128, L), np.float32)
    d_end = np.ones(n_e, bool)
    d_end[:-1] = start[1:]

    iP[cd, pd, t] = srcrow >> 1
    par[cd, pd, t] = (srcrow & 1).astype(np.float32)
    mask_f[cd, pd, t] = (~start).astype(np.float32)
    mask_r[cd, pd, t] = (~d_end).astype(np.float32)
    mask_v[cd, pd, t] = 1.0
    # pad slots keep mask_f = mask_r = mask_v = 0 and iP -> own core's DUM pair
    for k in range(M):
        pad = mask_v[k] == 0.0
        iP[k][pad] = (k * NPC + DUM) >> 1

    # quad xr index: quad (p, qd) -> local dst row of the segment covering it
    iXq = np.full((M, 128, LQ), DUM, np.int64)
    dls = ds - cd * npc
    # every quad within a segment has a real edge at its first slot
    sel = (q % 4) == 0
    iXq[cd[sel], pd[sel], (t[sel] // 4)] = dls[sel]

    def to_wrap(a):  # [128, X] -> wrap over i = col*128 + p sequence
        seq = a.T.reshape(-1)
        return _wrap_idx(seq)

    iP_w = np.stack([to_wrap(iP[k]) for k in range(M)])
    iX_w = np.stack([to_wrap(iXq[k]) for k in range(M)])

    # ---- head constants (de-permuted / de-scaled) ----
    A = bn_gamma.astype(np.float64) / np.sqrt(bn_var.astype(np.float64) + BN_EPS)
    Ap = (A[perm] / (N * absv)).astype(np.float32).reshape(F, 1)
    Bp = ((bias1 - bn_mean).astype(np.float64) * A + bn_beta)[perm] \
        .astype(np.float32).reshape(F, 1)
    Wcp = Wc[perm, :].astype(np.float32)

    # ---- per-core x^T (padded, bf16) ----
    xT = np.zeros((M, 128, NPC), BF16)
    for k in range(M):
        xT[k, :, :npc] = x[k * npc:(k + 1) * npc].T.astype(BF16)

    meta = dict(NPC=NPC, CH=CH, NG=NG, L=L, PP=PP, N=N, DUM=DUM, NCH=nch)

    in_maps = []
    for k in range(M):
        in_maps.append({
            "xT": np.ascontiguousarray(xT[k]),
            "Wgl": Wg_l.astype(BF16),
            "bgl": bg_l.reshape(1, F).astype(BF16),
            "Wgr": Wg_r.astype(BF16),
            "bgr": bg_r.reshape(1, F).astype(BF16),
            "iP": np.ascontiguousarray(iP_w[k]),
            "iX": np.ascontiguousarray(iX_w[k]),
            "par": np.ascontiguousarray(par[k]),
            "mask_f": np.ascontiguousarray(mask_f[k]),
            "mask_r": np.ascontiguousarray(mask_r[k]),
            "mask_v": np.ascontiguousarray(mask_v[k]),
            "Ap": Ap,
            "Bp": Bp,
            "Wcp": Wcp,
            "bc": bc.reshape(1, NCLS).astype(np.float32),
        })
    return in_maps, meta


def build(meta):
    NPC, CH, NG, L, PP, DUM, NCH = (
        meta[k] for k in ("NPC", "CH", "NG", "L", "PP", "DUM", "NCH"))
    LQ = L // 4
    NI = KC * 128           # pair-gather indices per chunk
    NIQ = (KC // 4) * 128   # quad-gather indices per chunk
    LW = (L * 128) // 16
    LWQ = (LQ * 128) // 16
    W2 = 2 * F * KC

    dt = mybir.dt
    alu = mybir.AluOpType
    act = mybir.ActivationFunctionType
    rg = [list(range(M))]

    nc = bacc.Bacc("TRN2", target_bir_lowering=False, debug=False, num_devices=M)

    def p_in(name, shape, d):
        return nc.dram_tensor(name, shape, d, kind="ExternalInput").ap()

    xT = p_in("xT", [128, NPC], dt.bfloat16)
    Wgl = p_in("Wgl", [F, F], dt.bfloat16)
    bgl = p_in("bgl", [1, F], dt.bfloat16)
    Wgr = p_in("Wgr", [F, F], dt.bfloat16)
    bgr = p_in("bgr", [1, F], dt.bfloat16)
    iP = p_in("iP", [128, LW], dt.int16)
    iX = p_in("iX", [128, LWQ], dt.int16)
    par = p_in("par", [128, L], dt.float32)
    mask_f = p_in("mask_f", [128, L], dt.float32)
    mask_r = p_in("mask_r", [128, L], dt.float32)
    mask_v = p_in("mask_v", [128, L], dt.float32)
    Ap = p_in("Ap", [F, 1], dt.float32)
    Bp = p_in("Bp", [F, 1], dt.float32)
    Wcp = p_in("Wcp", [F, NCLS], dt.float32)
    bc = p_in("bc", [1, NCLS], dt.float32)
    out = nc.dram_tensor("out", [1, NCLS], dt.float32, kind="ExternalOutput").ap()

    with tile.TileContext(nc) as tc:
        with (
            tc.tile_pool(name="dram", bufs=1, space="DRAM") as dpool,
            tc.tile_pool(name="sbp", bufs=1) as sbp,
            tc.tile_pool(name="sbw", bufs=2) as sbw,
            tc.tile_pool(name="ps2", bufs=2, space="PSUM") as pp,
            tc.tile_pool(name="ps1", bufs=1, space="PSUM") as pp1,
        ):
            xg_loc = dpool.tile([NPC, F], dt.bfloat16)
            xr_mini = dpool.tile([NPC, F], dt.bfloat16)
            xrq = dpool.tile([NPC, 8 * F], dt.bfloat16)
            xg_full = dpool.tile([NG, F], dt.bfloat16, addr_space="Shared")
            part_loc = dpool.tile([1, F], dt.float32)
            pooled = dpool.tile([1, F], dt.float32, addr_space="Shared")

            # ---- persistent SBUF ----
            xT_sb = sbp.tile([128, NPC], dt.bfloat16)
            nc.sync.dma_start(xT_sb[:], xT)
            wt = {}
            for nm, apin, sh in (("Wgl", Wgl, [F, F]), ("bgl", bgl, [1, F]),
                                 ("Wgr", Wgr, [F, F]), ("bgr", bgr, [1, F])):
                tl = sbp.tile(sh, dt.bfloat16, tag=nm)
                nc.sync.dma_start(tl[:], apin)
                wt[nm] = tl
            ones_sb = sbp.tile([1, F], dt.bfloat16)
            nc.vector.memset(ones_sb[:], 1.0)

            # ---- stage A: xl table + AllGather first (gates the pair gathers);
            # xr table + quad-dup built while the AllGather runs.  Table rows
            # are staged 4 matmul-chunks at a time so each DRAM write is one
            # big DMA instead of 49 small ones (sync-queue dispatch is ~1us). ----
            def build_table(wn, bn_, dst_dram):
                for g0 in range(0, CH, 4):
                    gn = min(4, CH - g0)
                    stg = sbw.tile([128, 4 * F], dt.bfloat16, tag="stg")
                    for j in range(gn):
                        ci = g0 + j
                        lhs = xT_sb[:, 128 * ci:128 * (ci + 1)]
                        ps = pp.tile([128, F], dt.float32, tag="psA")
                        nc.tensor.matmul(ps[:], lhsT=lhs, rhs=wt[wn][:],
                                         start=True, stop=False)
                        nc.tensor.matmul(ps[:], lhsT=ones_sb[:], rhs=wt[bn_][:],
                                         start=False, stop=True)
                        nc.vector.tensor_copy(stg[:, F * j:F * (j + 1)], ps[:])
                    nc.sync.dma_start(
                        dst_dram[128 * g0:128 * (g0 + gn), :]
                        .rearrange("(c p) f -> p c f", c=gn),
                        stg[:, 0:gn * F].rearrange("p (c f) -> p c f", c=gn))

            build_table("Wgl", "bgl", xg_loc)
            nc.gpsimd.collective_compute(
                "AllGather", alu.bypass, replica_groups=rg,
                ins=[xg_loc.opt()], outs=[xg_full.opt()])

            build_table("Wgr", "bgr", xr_mini)
            # duplicate xr rows 8x: quad table row j = [xr_j]*8
            for i in range(8):
                nc.sync.dma_start(
                    xrq[:].rearrange("a (e f) -> a e f", e=8)[:, i, :], xr_mini[:])

            # grid metadata loads: only needed once chunk-0 compute starts
            iP_sb = sbp.tile([128, LW], dt.int16)
            nc.sync.dma_start(iP_sb[:], iP)
            iX_sb = sbp.tile([128, LWQ], dt.int16)
            nc.sync.dma_start(iX_sb[:], iX)
            par_sb = sbp.tile([128, L], dt.float32)
            nc.sync.dma_start(par_sb[:], par)
            mf_sb = sbp.tile([128, L], dt.float32)
            nc.sync.dma_start(mf_sb[:], mask_f)
            mr_sb = sbp.tile([128, L], dt.float32)
            nc.sync.dma_start(mr_sb[:], mask_r)
            mv_sb = sbp.tile([128, L], dt.float32)
            nc.sync.dma_start(mv_sb[:], mask_v)

            tab_pair = xg_full[:].rearrange("(a two) f -> a (two f)", two=2)

            # ---- fused pass over chunks ----
            def gather_pair(dst_tile, c):
                nc.gpsimd.dma_gather(
                    out_ap=dst_tile[:].rearrange("p (b f) -> p b f", f=2 * F),
                    in_ap=tab_pair,
                    idxs_ap=iP_sb[:, (NI // 16) * c:(NI // 16) * (c + 1)],
                    num_idxs=NI, num_idxs_reg=NI, elem_size=2 * F,
                    single_packet=False)

            def gather_quad(dst_tile, c):
                nc.gpsimd.dma_gather(
                    out_ap=dst_tile[:].rearrange("p (b f) -> p b f", f=8 * F),
                    in_ap=xrq[:],
                    idxs_ap=iX_sb[:, (NIQ // 16) * c:(NIQ // 16) * (c + 1)],
                    num_idxs=NIQ, num_idxs_reg=NIQ, elem_size=8 * F,
                    single_packet=False)

            LEAD = min(2, NCH)
            gx_tiles = {}
            for c in range(LEAD):
                gx = sbw.tile([128, W2], dt.bfloat16, tag="gx", bufs=3)
                gather_quad(gx, c)
                gx_tiles[c] = gx

            pacc = pp1.tile([F, 1], dt.float32, tag="pacc")
            for c in range(NCH):
                sl = slice(c * KC, (c + 1) * KC)
                gp = sbw.tile([128, W2], dt.bfloat16, tag="gp", bufs=3)
                gather_pair(gp, c)
                if c + LEAD < NCH:
                    gxn = sbw.tile([128, W2], dt.bfloat16, tag="gx", bufs=3)
                    gather_quad(gxn, c + LEAD)
                    gx_tiles[c + LEAD] = gxn
                gx = gx_tiles.pop(c)

                # z = gl + gr ; lrelu(z) = max(z, 0.2 z)   (all in place)
                nc.vector.tensor_tensor(out=gx[:], in0=gx[:], in1=gp[:],
                                        op=alu.add)
                nc.vector.scalar_tensor_tensor(
                    out=gx[:], in0=gx[:], scalar=0.2, in1=gx[:],
                    op0=alu.mult, op1=alu.max)
                m3 = gx[:].rearrange("p (k f) -> p k f", f=2 * F)
                lgp = sbw.tile([128, KC], dt.float32, tag="lgp")
                lgn = sbw.tile([128, KC], dt.float32, tag="lgn")
                lgp1 = sbw.tile([128, KC], dt.float32, tag="lgp1")
                lgn1 = sbw.tile([128, KC], dt.float32, tag="lgn1")
                nc.vector.tensor_reduce(lgp[:], m3[:, :, 0:PP],
                                        axis=mybir.AxisListType.X, op=alu.add)
                nc.vector.tensor_reduce(lgn[:], m3[:, :, PP:F],
                                        axis=mybir.AxisListType.X, op=alu.add)
                nc.vector.tensor_reduce(lgp1[:], m3[:, :, F:F + PP],
                                        axis=mybir.AxisListType.X, op=alu.add)
                nc.vector.tensor_reduce(lgn1[:], m3[:, :, F + PP:2 * F],
                                        axis=mybir.AxisListType.X, op=alu.add)
                # a = p0-n0 ; b = p1-n1 ; logit = a + par*(b-a)
                nc.vector.tensor_tensor(out=lgp[:], in0=lgp[:], in1=lgn[:],
                                        op=alu.subtract)
                nc.vector.tensor_tensor(out=lgp1[:], in0=lgp1[:], in1=lgn1[:],
                                        op=alu.subtract)
                nc.vector.tensor_tensor(out=lgp1[:], in0=lgp1[:], in1=lgp[:],
                                        op=alu.subtract)
                nc.vector.tensor_tensor(out=lgp1[:], in0=lgp1[:],
                                        in1=par_sb[:, sl], op=alu.mult)
                nc.vector.tensor_tensor(out=lgn[:], in0=lgp[:],
                                        in1=lgp1[:], op=alu.add)

                # chunk-local segment softmax
                E = sbw.tile([128, KC], dt.float32, tag="E")
                S = sbw.tile([128, KC], dt.float32, tag="S")
                D = sbw.tile([128, KC], dt.float32, tag="D")
                nc.scalar.activation(E[:], lgn[:], act.Exp)
                nc.vector.tensor_tensor_scan(
                    out=S[:], data0=mf_sb[:, sl], data1=E[:], initial=0.0,
                    op0=alu.mult, op1=alu.add)
                nc.vector.tensor_tensor_scan(
                    out=D[:, ::-1], data0=mr_sb[:, sl][:, ::-1],
                    data1=S[:, ::-1], initial=0.0, op0=alu.mult, op1=alu.max)
                nc.vector.reciprocal(D[:], D[:])
                nc.vector.tensor_tensor(out=E[:], in0=E[:], in1=D[:],
                                        op=alu.mult)
                nc.vector.tensor_tensor(out=E[:], in0=E[:], in1=mv_sb[:, sl],
                                        op=alu.mult)
                # w1 = w*par ; w0 = w - w1
                nc.vector.tensor_tensor(out=S[:], in0=E[:], in1=par_sb[:, sl],
                                        op=alu.mult)
                nc.vector.tensor_tensor(out=D[:], in0=E[:], in1=S[:],
                                        op=alu.subtract)
                wb0 = sbw.tile([128, KC], dt.bfloat16, tag="wb0")
                wb1 = sbw.tile([128, KC], dt.bfloat16, tag="wb1")
                nc.vector.tensor_copy(wb0[:], D[:])
                nc.vector.tensor_copy(wb1[:], S[:])

                # weighted sum while gp is in SBUF
                for b in range(KC):
                    nc.tensor.matmul(
                        pacc[:], lhsT=gp[:, 2 * F * b:2 * F * b + F],
                        rhs=wb0[:, b:b + 1], start=(c == 0 and b == 0),
                        stop=False)
                    nc.tensor.matmul(
                        pacc[:], lhsT=gp[:, 2 * F * b + F:2 * F * (b + 1)],
                        rhs=wb1[:, b:b + 1], start=False,
                        stop=(c == NCH - 1 and b == KC - 1))

            part_sb = sbp.tile([F, 1], dt.float32)
            nc.vector.tensor_copy(part_sb[:], pacc[:])
            nc.sync.dma_start(part_loc[:].rearrange("o f -> f o"), part_sb[:])

            nc.gpsimd.collective_compute(
                "AllReduce", alu.add, replica_groups=rg,
                ins=[part_loc.opt()], outs=[pooled.opt()])

            # ---- head ----
            pool_sb = sbp.tile([F, 1], dt.float32)
            nc.sync.dma_start(pool_sb[:], pooled[:].rearrange("o f -> f o"))
            Ap_sb = sbp.tile([F, 1], dt.float32)
            nc.sync.dma_start(Ap_sb[:], Ap)
            Bp_sb = sbp.tile([F, 1], dt.float32)
            nc.sync.dma_start(Bp_sb[:], Bp)
            Wc_sb = sbp.tile([F, NCLS], dt.float32)
            nc.sync.dma_start(Wc_sb[:], Wcp)
            bc_sb = sbp.tile([1, NCLS], dt.float32)
            nc.sync.dma_start(bc_sb[:], bc)
            h_sb = sbp.tile([F, 1], dt.float32)
            nc.vector.scalar_tensor_tensor(
                out=h_sb[:], in0=pool_sb[:], scalar=Ap_sb[:, 0:1], in1=Bp_sb[:],
                op0=alu.mult, op1=alu.add)
            one1 = sbp.tile([1, 1], dt.float32)
            nc.vector.memset(one1[:], 1.0)
            hp = pp1.tile([1, NCLS], dt.float32, tag="hp")
            nc.tensor.matmul(hp[:], lhsT=h_sb[:], rhs=Wc_sb[:], start=True,
                             stop=False)
            nc.tensor.matmul(hp[:], lhsT=one1[:], rhs=bc_sb[:], start=False,
                             stop=True)
            eh = sbp.tile([1, NCLS], dt.float32)
            nc.scalar.activation(eh[:], hp[:], act.Exp)
            den = sbp.tile([1, 1], dt.float32)
            nc.vector.tensor_reduce(den[:], eh[:], axis=mybir.AxisListType.X,
                                    op=alu.add)
            rden = sbp.tile([1, 1], dt.float32)
            nc.vector.reciprocal(rden[:], den[:])
            osb = sbp.tile([1, NCLS], dt.float32)
            nc.vector.tensor_scalar(out=osb[:], in0=eh[:], scalar1=rden[:, 0:1],
                                    scalar2=None, op0=alu.mult)
            nc.sync.dma_start(out, osb[:])

    nc.compile()
    return nc


# --------------------------------------------------------------------------
# public entry point
# --------------------------------------------------------------------------

_CACHE = {}


def _install_ntff_hook():
    """Provide antenv.axon_hooks + the ctypes NTFF hook when the image lacks
    them, so run_bass_kernel_spmd(trace=True) can capture exec_time_ns."""
    import contextlib
    import ctypes
    import sys
    import types

    try:
        import antenv.axon_hooks  # noqa: F401
        return
    except ImportError:
        pass
    try:
        import antenv
    except ImportError:
        return
    holder = [None]
    mod = types.ModuleType("antenv.axon_hooks")
    mod.set_axon_ntff_profile_hook = lambda h: holder.__setitem__(0, h)
    mod.get_axon_ntff_profile_hook = lambda: holder[0]
    sys.modules["antenv.axon_hooks"] = mod
    antenv.axon_hooks = mod

    so_path = "/opt/axon/libaxon_pjrt.so"
    if os.path.exists(so_path):
        lib = ctypes.CDLL(so_path)
        if hasattr(lib, "axon_start_nrt_profile"):
            lib.axon_start_nrt_profile.argtypes = [
                ctypes.POINTER(ctypes.c_int64), ctypes.c_size_t]
            lib.axon_start_nrt_profile.restype = ctypes.c_int64
            lib.axon_stop_nrt_profile.argtypes = [ctypes.c_char_p]
            lib.axon_stop_nrt_profile.restype = ctypes.c_int64

            @contextlib.contextmanager
            def _hook(output_dir, device_ids):
                import jax
                jax.devices()
                if device_ids:
                    ids = (ctypes.c_int64 * len(device_ids))(*device_ids)
                    rc = lib.axon_start_nrt_profile(ids, len(device_ids))
                else:
                    rc = lib.axon_start_nrt_profile(None, 0)
                if rc != 0:
                    raise RuntimeError(f"axon_start_nrt_profile rc={rc}")
                try:
                    yield
                finally:
                    n = lib.axon_stop_nrt_profile(str(output_dir).encode())
                    print(f"ntff profile: {n} file(s) -> {output_dir}")

            mod.set_axon_ntff_profile_hook(_hook)

    import concourse.bass_utils as bu
    bu.upload_artifacts = lambda tmpdir: "local://" + str(tmpdir)


def kernel(**inputs):
    from concourse.bass_utils import run_bass_kernel_spmd

    if bool(int(os.environ.get("KERNEL_TRACE", "0"))):
        _install_ntff_hook()
    inputs = {k: np.asarray(v) for k, v in inputs.items()}
    in_maps, meta = prep_host(**inputs)
    key = tuple(sorted(meta.items()))
    if key not in _CACHE:
        _CACHE[key] = build(meta)
    nc = _CACHE[key]
    res = run_bass_kernel_spmd(nc, in_maps, core_ids=list(range(M)),
                               trace=bool(int(os.environ.get("KERNEL_TRACE", "0"))))
    if getattr(res, "exec_time_ns", None) is not None:
        print(f"HW exec time: {res.exec_time_ns} ns")
    return np.asarray(res.results[0]["out"]).astype(np.float32)
